# revision 16
# baseline (speedup 1.0000x reference)
# Bass/Tile TRN2 kernel for nn_BiasedCrossDecoderLayer (dense cross-attention
# transformer decoder layer), SPMD over 8 NeuronCores.
#
# Sharding: core c -> batch b = c//4, head-group hg = c%4 (4 of 16 heads).
# Attention is head-parallel and processed in two query chunks of 512; after
# each chunk the out-projection partial sums are ReduceScattered (bf16) within
# the 4-core batch group so the collective overlaps the next chunk's compute.
# Query ownership is interleaved: core c owns queries [128c,128c+128) of chunk
# A and [512+128c, 512+128c+128) of chunk B (the host gather restores order),
# so each chunk's RS delivers a 128-query piece directly.  The FFN then runs
# sequence-parallel on the core's 256 owned queries with the full 4096 hidden.
#
# LayerNorms are folded into the weights host-side (bf16 weights; rank-2
# [mean;std] correction matmuls in fp32r appended to each PSUM group).  The
# attention mask is added to the logits by an identity-matmul into the same
# PSUM accumulation group (no DVE pass); softmax denominators come from a
# ones-column appended to V (M=65 PV matmul).  The whole heavy datapath is
# bf16 (weights, activations, mask, probs) which halves DMA traffic and
# enables fast weight loads; PSUM accumulation stays fp32.  QK logits for a
# head pair run concurrently in two 64-row PE groups (K=64 row tiling).
#
# The FFN weights (w1 AND w2, bf16) are DMA'd into SBUF during attention so
# ff1/ff2 run back-to-back per hidden block with zero weight stalls.

import os
import sys

import numpy as np

sys.path.insert(0, "/opt/trn_rl_repo")

import ml_dtypes  # noqa: E402

import concourse.bass as bass  # noqa: E402
import concourse.mybir as mybir  # noqa: E402
import concourse.tile as tile  # noqa: E402
from concourse import bacc  # noqa: E402

F32 = mybir.dt.float32
F32R = mybir.dt.float32r
BF16 = mybir.dt.bfloat16
AF = mybir.ActivationFunctionType
ALU = mybir.AluOpType

B, Q, S, D, H = 2, 1024, 2048, 1024, 16
HD = D // H       # 64
FF = 4 * D
EPS = 1e-5
NCORES = 8
NH = 4            # heads per core
FC = NH * HD      # 256 qkv feature dims per core
QS = 256          # queries owned per core (two 128-query pieces)
QC = 512          # attention query chunk
P = 128
KX = D // P       # 8 k-tiles over the model dim
FFP = FF // P     # 32 hidden blocks
NST = S // P      # 16 s-tiles

REPLICA_GROUPS = [[0, 1, 2, 3], [4, 5, 6, 7]]

LAST_RESULT = None  # BassKernelResults of the most recent run (for test.py)


def _r(ap):
    return ap.bitcast(F32R)


def _f(ap):
    return ap.bitcast(F32)


def build_nc():
    nc = bacc.Bacc(
        "TRN2",
        target_bir_lowering=False,
        debug=False,
        num_devices=NCORES,
        name="biased_cross_decoder",
    )

    d = {}
    d["ones_t"] = nc.dram_tensor("ones_t", [P, P], F32R, kind="ExternalInput").ap()
    d["cb"] = nc.dram_tensor("cb", [P, P + 64], BF16, kind="ExternalInput").ap()
    d["xT"] = nc.dram_tensor("xT", [D, Q], BF16, kind="ExternalInput").ap()
    d["zT"] = nc.dram_tensor("zT", [D, S], BF16, kind="ExternalInput").ap()
    d["xq"] = nc.dram_tensor("xq", [D, QS], F32, kind="ExternalInput").ap()
    d["maskT"] = nc.dram_tensor("maskT", [2, 2, S, 2, QC], BF16,
                                kind="ExternalInput").ap()
    d["wqT"] = nc.dram_tensor("wqT", [P, KX, FC], BF16, kind="ExternalInput").ap()
    d["wkT"] = nc.dram_tensor("wkT", [P, KX, FC], BF16, kind="ExternalInput").ap()
    d["wvT"] = nc.dram_tensor("wvT", [P, KX, FC], BF16, kind="ExternalInput").ap()
    d["adjq"] = nc.dram_tensor("adjq", [2, FC], F32R, kind="ExternalInput").ap()
    d["adjk"] = nc.dram_tensor("adjk", [2, FC], F32R, kind="ExternalInput").ap()
    d["adjv"] = nc.dram_tensor("adjv", [2, FC], F32R, kind="ExternalInput").ap()
    d["owp"] = nc.dram_tensor("owp", [P, 2, D], BF16, kind="ExternalInput").ap()
    d["outb"] = nc.dram_tensor("outb", [D], F32, kind="ExternalInput").ap()
    d["b1c"] = nc.dram_tensor("b1c", [P, FFP], F32, kind="ExternalInput").ap()
    d["b2"] = nc.dram_tensor("b2", [D], F32, kind="ExternalInput").ap()
    d["w1p"] = nc.dram_tensor("w1p", [FFP, P, KX, P], BF16,
                              kind="ExternalInput").ap()
    d["w2T"] = nc.dram_tensor("w2T", [FF, D], BF16, kind="ExternalInput").ap()
    d["out"] = nc.dram_tensor("out", [D, QS], F32, kind="ExternalOutput").ap()

    with tile.TileContext(nc) as tc:
        build_tile_program(tc, nc, d)
    nc.compile()
    return nc


class _Pool:
    """Keeps the tile_pool context manager alive; allows explicit close."""

    def __init__(self, cm):
        self._cm = cm
        self.pool = cm.__enter__()

    def tile(self, *a, **kw):
        kw.setdefault("name", kw.get("tag") or "t")
        return self.pool.tile(*a, **kw)

    def close(self):
        self._cm.__exit__(None, None, None)


def build_tile_program(tc, nc, d):
    # ---------------- persistent constants ----------------
    const = _Pool(tc.tile_pool(name="const", bufs=1))
    dram = _Pool(tc.tile_pool(name="dram", bufs=1, space="DRAM"))

    ones_sb = const.tile([P, P], F32R, tag="ones_sb")
    nc.sync.dma_start(ones_sb, d["ones_t"])
    cb_sb = const.tile([P, P + 64], BF16, tag="cb_sb")
    nc.sync.dma_start(cb_sb, d["cb"])
    ident = cb_sb[:, 0:P]            # bf16 identity (mask-add matmul lhsT)
    ones_bcol = cb_sb[:, P:P + 1]    # bf16 ones column (bf16 stat sums)
    ones_col = ones_sb[:, 0:1]       # f32r ones column (fp32 stat sums)
    ones_row = ones_sb[0:1, :]       # f32r row (partition broadcasts)

    eps_t = const.tile([1, 1], F32, tag="eps")
    nc.vector.memset(eps_t, EPS)
    outb_col = const.tile([P, KX], F32, tag="outb_col")
    nc.sync.dma_start(outb_col, d["outb"].rearrange("(o p) -> p o", p=P))
    b2_col = const.tile([P, KX], F32, tag="b2_col")
    nc.sync.dma_start(b2_col, d["b2"].rearrange("(o p) -> p o", p=P))
    b1_col = const.tile([P, FFP], F32, tag="b1_col")
    nc.sync.dma_start(b1_col, d["b1c"])
    rz_col = const.tile([P, NST], F32R, tag="rz_col")

    rs_scr = dram.tile([1, S], F32R, tag="rs_scr")
    rs_in = [dram.tile([4, D, P], BF16, name=f"rs_in{i}", tag=f"rs_in{i}")
             for i in range(2)]
    rs_out = [dram.tile([D, P], BF16, name=f"rs_out{i}", tag=f"rs_out{i}")
              for i in range(2)]

    # ---------------- long-lived right-side pools ----------------
    pool_qkv = _Pool(tc.tile_pool(name="qkv", bufs=1, side="right"))
    qT = pool_qkv.tile([P, 2, Q], BF16, tag="qT")
    kT = pool_qkv.tile([P, 2, S], BF16, tag="kT")
    v_sb = pool_qkv.tile([P, NST, NH, HD + 1], BF16, tag="v_sb")
    ow_sb = pool_qkv.tile([P, 2, D], BF16, tag="ow_sb")
    nc.sync.dma_start(ow_sb, d["owp"])

    # ---------------- phase A scratch (left stack) ----------------
    pool_x = _Pool(tc.tile_pool(name="px", bufs=1))
    pool_z = _Pool(tc.tile_pool(name="pz", bufs=1))
    pool_w = _Pool(tc.tile_pool(name="pw", bufs=1))
    pool_adj = _Pool(tc.tile_pool(name="adj", bufs=1))
    pool_bc = _Pool(tc.tile_pool(name="bc", bufs=1))
    pool_sq = _Pool(tc.tile_pool(name="sq", bufs=3))
    pool_rows = _Pool(tc.tile_pool(name="rows", bufs=2))

    xT = pool_x.tile([P, KX, Q], BF16, tag="xT")
    for k in range(KX):
        nc.sync.dma_start(xT[:, k, :], d["xT"][k * P:(k + 1) * P, :])
    zT = pool_z.tile([P, KX, S], BF16, tag="zT")
    for h2 in range(2):
        for k in range(KX):
            nc.sync.dma_start(zT[:, k, h2 * 1024:(h2 + 1) * 1024],
                              d["zT"][k * P:(k + 1) * P, h2 * 1024:(h2 + 1) * 1024])
    wq_sb = pool_w.tile([P, KX, FC], BF16, tag="wq_sb")
    nc.sync.dma_start(wq_sb, d["wqT"])
    wk_sb = pool_w.tile([P, KX, FC], BF16, tag="wk_sb")
    nc.sync.dma_start(wk_sb, d["wkT"])
    wv_sb = pool_w.tile([P, KX, FC], BF16, tag="wv_sb")
    nc.sync.dma_start(wv_sb, d["wvT"])
    adjq_w = pool_w.tile([2, FC], F32R, tag="adjq_w")
    nc.sync.dma_start(adjq_w, d["adjq"])
    adjk_w = pool_w.tile([2, FC], F32R, tag="adjk_w")
    nc.sync.dma_start(adjk_w, d["adjk"])
    adjv_w = pool_w.tile([2, FC], F32R, tag="adjv_w")
    nc.sync.dma_start(adjv_w, d["adjv"])

    adjx = pool_adj.tile([2, Q], F32R, tag="adjx")      # [mean ; std] rows
    adjz = pool_adj.tile([2, S], F32R, tag="adjz")
    rxB = pool_bc.tile([P, Q], F32, tag="rxB")          # 1/std broadcast
    rzB = pool_bc.tile([P, S], F32, tag="rzB")

    def ln_stats(aT, T, adj, rB, ps_stats, scr=None):
        """Per 512-token chunk: LN stats -> adj=[mean;std] rows and a
        [P, T] broadcast of 1/std (via gpsimd partition_broadcast)."""
        for ch in range(T // 512):
            sl = slice(ch * 512, (ch + 1) * 512)
            ps_sum = ps_stats.tile([1, 512], F32, name="ps_sum", tag="ps_sum")
            ps_ssq = ps_stats.tile([1, 512], F32, name="ps_ssq", tag="ps_ssq")
            for k in range(KX):
                nc.tensor.matmul(ps_sum, ones_bcol, aT[:, k, sl],
                                 start=(k == 0), stop=(k == KX - 1))
                sq = pool_sq.tile([P, 512], BF16, name="sq", tag="sq")
                nc.scalar.square(sq, aT[:, k, sl])
                nc.tensor.matmul(ps_ssq, ones_bcol, sq,
                                 start=(k == 0), stop=(k == KX - 1))
            e2 = pool_rows.tile([1, 512], F32, name="e2", tag="e2")
            m2 = pool_rows.tile([1, 512], F32, name="m2", tag="m2")
            inv = pool_rows.tile([1, 512], F32R, name="inv", tag="inv")
            rr = pool_rows.tile([1, 512], F32R, name="rr", tag="rr")
            nc.vector.tensor_scalar_mul(adj[0:1, sl], ps_sum, 1.0 / D)  # mean
            nc.vector.tensor_scalar_mul(e2, ps_ssq, 1.0 / D)            # E[x^2]
            nc.vector.tensor_mul(m2, adj[0:1, sl], adj[0:1, sl])
            nc.vector.tensor_sub(e2, e2, m2)                            # var
            nc.scalar.activation(inv, e2, AF.Sqrt, bias=eps_t[0:1])     # std
            nc.vector.reciprocal_approx_fast(_f(rr), _f(inv))
            nc.scalar.dma_start(adj[1:2, sl], inv)   # cross-partition row move
            nc.gpsimd.partition_broadcast(rB[:, sl], _f(rr))
            if scr is not None:
                nc.scalar.dma_start(scr[0:1, sl], rr)

    # ---- x and z statistics (dense on PE; post-chains hide underneath) ----
    with tc.tile_pool(name="ps_sx", bufs=2, space="PSUM") as ps_sx:
        ln_stats(xT, Q, adjx, rxB, ps_sx)
    with tc.tile_pool(name="ps_sz", bufs=2, space="PSUM") as ps_sz:
        ln_stats(zT, S, adjz, rzB, ps_sz, scr=rs_scr)
    nc.scalar.dma_start(rz_col, rs_scr.rearrange("a (i p) -> (a p) i", p=P))

    # ---- q projection ----
    with tc.tile_pool(name="ps_q", bufs=3, space="PSUM") as ps_qk:
        for m in range(2):
            for ch in range(2):
                sl = slice(ch * 512, (ch + 1) * 512)
                ps = ps_qk.tile([P, 512], F32, name="ps_qk_t", tag="ps_qk_t")
                for k in range(KX):
                    nc.tensor.matmul(ps, wq_sb[:, k, m * P:(m + 1) * P],
                                     xT[:, k, sl], start=(k == 0), stop=False)
                nc.tensor.matmul(ps, adjq_w[:, m * P:(m + 1) * P],
                                 _r(adjx[:, sl]), start=False, stop=True)
                nc.vector.tensor_mul(qT[:, m, sl], ps, rxB[:, sl])

    # ---- k/v projections ----
    # softmax-denominator ones column
    nc.sync.dma_start(
        v_sb[:, :, :, HD:HD + 1],
        d["cb"][:, P:P + 64].rearrange("p (a b c) -> p a b c", a=NST, c=1))

    with tc.tile_pool(name="ps_k", bufs=3, space="PSUM") as ps_qk, \
         tc.tile_pool(name="ps_v", bufs=2, space="PSUM") as ps_v:
        for m in range(2):
            for ch in range(4):
                sl = slice(ch * 512, (ch + 1) * 512)
                ps = ps_qk.tile([P, 512], F32, name="ps_qk_t", tag="ps_qk_t")
                for k in range(KX):
                    nc.tensor.matmul(ps, wk_sb[:, k, m * P:(m + 1) * P],
                                     zT[:, k, sl], start=(k == 0), stop=False)
                nc.tensor.matmul(ps, adjk_w[:, m * P:(m + 1) * P],
                                 _r(adjz[:, sl]), start=False, stop=True)
                nc.vector.tensor_mul(kT[:, m, sl], ps, rzB[:, sl])

        for t in range(NST):
            ps = ps_v.tile([P, FC], F32, name="ps_v_t", tag="ps_v_t")
            for k in range(KX):
                nc.tensor.matmul(ps, zT[:, k, t * P:(t + 1) * P],
                                 wv_sb[:, k, :], start=(k == 0), stop=False)
            nc.tensor.matmul(ps, _r(adjz[:, t * P:(t + 1) * P]), _r(adjv_w),
                             start=False, stop=True)
            nc.vector.tensor_scalar_mul(
                v_sb[:, t, :, 0:HD],
                ps.rearrange("p (h e) -> p h e", h=NH),
                _f(rz_col[:, t:t + 1]))

    pool_rows.close()
    pool_sq.close()
    pool_bc.close()
    pool_adj.close()
    pool_w.close()
    pool_z.close()
    pool_x.close()

    # ---------------- resident FFN weights (prefetched during attention) ----
    pool_w1 = _Pool(tc.tile_pool(name="w1r", bufs=1))
    w1sb = pool_w1.tile([P, FFP, KX, P], BF16, tag="w1sb")
    w2sb = pool_w1.tile([P, FFP, D], BF16, tag="w2sb")
    for j0 in range(0, FFP, 8):
        nc.gpsimd.dma_start(
            w1sb[:, j0:j0 + 8],
            d["w1p"][j0:j0 + 8].rearrange("j p k o -> p j k o"))
        nc.gpsimd.dma_start(
            w2sb[:, j0:j0 + 8],
            d["w2T"][j0 * P:(j0 + 8) * P, :].rearrange("(j p) o -> p j o", p=P))

    # =================== attention (query-chunked) ===================
    pool_att2 = _Pool(tc.tile_pool(name="att2", bufs=1))
    pool_mk = _Pool(tc.tile_pool(name="mk", bufs=9))
    pool_pr0 = _Pool(tc.tile_pool(name="pr0", bufs=3))
    pool_pr1 = _Pool(tc.tile_pool(name="pr1", bufs=3))
    pool_nrm = _Pool(tc.tile_pool(name="nrm", bufs=2))
    pool_osb = _Pool(tc.tile_pool(name="osb", bufs=3))

    for ci in range(2):
        qsl = slice(ci * QC, (ci + 1) * QC)
        att2 = [pool_att2.tile([P, QC], BF16, name=f"att2_{p}", tag=f"att2_{p}")
                for p in range(2)]

        ps_lg0_cm = tc.tile_pool(name="ps_lg0", bufs=3, space="PSUM")
        ps_lg1_cm = tc.tile_pool(name="ps_lg1", bufs=3, space="PSUM")
        ps_att_cm = tc.tile_pool(name="ps_att", bufs=1, space="PSUM")
        ps_lg0 = ps_lg0_cm.__enter__()
        ps_lg1 = ps_lg1_cm.__enter__()
        ps_att = ps_att_cm.__enter__()

        for pair in range(2):
            attps = [ps_att.tile([HD + 1, QC], F32, name=f"attps{hh}",
                                 tag=f"attps{hh}") for hh in range(2)]

            def emit_pv(st, p0, p1):
                nc.tensor.matmul(attps[0], v_sb[:, st, 2 * pair, :], p0,
                                 start=(st == 0), stop=(st == NST - 1))
                nc.tensor.matmul(attps[1], v_sb[:, st, 2 * pair + 1, :], p1,
                                 start=(st == 0), stop=(st == NST - 1))

            prev = None
            for st in range(NST):
                ssl = slice(st * P, (st + 1) * P)
                mk = pool_mk.tile([P, 2, QC], BF16, name="mk", tag="mk")
                nc.sync.dma_start(mk, d["maskT"][ci, pair, ssl, :, :])
                lg0 = ps_lg0.tile([P, QC], F32, name="lg0", tag="lg0")
                lg1 = ps_lg1.tile([P, QC], F32, name="lg1", tag="lg1")
                # head-pair QK in two concurrent 64-row PE groups
                nc.tensor.matmul(lg0, kT[0:HD, pair, ssl], qT[0:HD, pair, qsl],
                                 start=True, stop=False)
                nc.tensor.matmul(lg1, kT[HD:P, pair, ssl], qT[HD:P, pair, qsl],
                                 start=True, stop=False)
                # mask add via identity matmul into the same PSUM group
                nc.tensor.matmul(lg0, ident, mk[:, 0, :], start=False, stop=True)
                nc.tensor.matmul(lg1, ident, mk[:, 1, :], start=False, stop=True)
                pr0 = pool_pr0.tile([P, QC], BF16, name="pr0", tag="pr0")
                nc.scalar.activation(pr0, lg0, AF.Exp)
                pr1 = pool_pr1.tile([P, QC], BF16, name="pr1", tag="pr1")
                nc.scalar.activation(pr1, lg1, AF.Exp)
                if prev is not None:
                    emit_pv(*prev)
                prev = (st, pr0, pr1)
            emit_pv(*prev)

            # normalize: att2[pair][64h:64h+64] = attps[h][0:64] / attps[h][64]
            for hh in range(2):
                den = pool_nrm.tile([1, QC], F32, name="den", tag="den")
                nc.vector.tensor_copy(den, attps[hh][HD:HD + 1, :])
                r0 = pool_nrm.tile([1, QC], F32, name="r0", tag="r0")
                nc.vector.reciprocal_approx_fast(r0, den)
                nbc = pool_nrm.tile([HD, QC], F32, name="nbc", tag="nbc")
                nc.gpsimd.partition_broadcast(nbc, r0)
                nc.vector.tensor_mul(att2[pair][HD * hh:HD * hh + HD, :],
                                     attps[hh][0:HD, :], nbc)

        ps_att_cm.__exit__(None, None, None)
        ps_lg1_cm.__exit__(None, None, None)
        ps_lg0_cm.__exit__(None, None, None)

        # ---- out-projection for this chunk + ReduceScatter ----
        with tc.tile_pool(name="ps_o", bufs=2, space="PSUM") as ps_o:
            for m in range(KX):
                ps = ps_o.tile([P, QC], F32, name="ps_o_t", tag="ps_o_t")
                nc.tensor.matmul(ps, ow_sb[:, 0, m * P:(m + 1) * P], att2[0],
                                 start=True, stop=False)
                nc.tensor.matmul(ps, ow_sb[:, 1, m * P:(m + 1) * P], att2[1],
                                 start=False, stop=True)
                ot = pool_osb.tile([P, QC], BF16, name="ot", tag="ot")
                nc.vector.tensor_copy(ot, ps)
                nc.sync.dma_start(
                    rs_in[ci][:, m * P:(m + 1) * P, :].rearrange(
                        "r p q -> p r q"),
                    ot.rearrange("p (r q) -> p r q", r=4))
        nc.gpsimd.collective_compute(
            "ReduceScatter",
            ALU.add,
            replica_groups=REPLICA_GROUPS,
            ins=[rs_in[ci].opt()],
            outs=[rs_out[ci].opt()],
        )

    pool_osb.close()
    pool_nrm.close()
    pool_pr1.close()
    pool_pr0.close()
    pool_mk.close()
    pool_att2.close()
    pool_qkv.close()

    # =================== residual + FFN (sequence-parallel) ===================
    pool_f = _Pool(tc.tile_pool(name="ffn", bufs=1, side="right"))
    pool_rsld = _Pool(tc.tile_pool(name="rsld", bufs=3))
    pool_fsq = _Pool(tc.tile_pool(name="fsq", bufs=2))
    pool_frow = _Pool(tc.tile_pool(name="frow", bufs=2))
    pool_ftmp = _Pool(tc.tile_pool(name="ftmp", bufs=2))
    pool_yo = _Pool(tc.tile_pool(name="yout", bufs=3))

    y1T = pool_f.tile([P, KX, QS], F32R, tag="y1T")
    y1n = pool_f.tile([P, KX, QS], BF16, tag="y1n")
    g_sb = pool_f.tile([P, FFP, QS], BF16, tag="g_sb")
    m_row = pool_f.tile([1, QS], F32R, tag="m_row")
    r_row = pool_f.tile([1, QS], F32, tag="r_row")
    r_rowR = pool_f.tile([1, QS], F32R, tag="r_rowR")
    myB = pool_f.tile([P, QS], F32, tag="myB")
    ryB = pool_f.tile([P, QS], F32, tag="ryB")
    xq_sb = pool_f.tile([P, KX, QS], F32, tag="xq_sb")
    nc.gpsimd.dma_start(xq_sb, d["xq"].rearrange("(k p) q -> p k q", p=P))

    def piece_prep(piece):
        """y1 assembly + LN stats + normalize for one 128-query piece."""
        psl = slice(piece * P, (piece + 1) * P)
        for k in range(KX):
            rst = pool_rsld.tile([P, P], BF16, name="rst", tag="rst")
            nc.gpsimd.dma_start(rst, rs_out[piece][k * P:(k + 1) * P, :])
            nc.vector.scalar_tensor_tensor(
                out=y1T[:, k, psl], in0=rst, scalar=outb_col[:, k:k + 1],
                in1=xq_sb[:, k, psl], op0=ALU.add, op1=ALU.add)
        with tc.tile_pool(name="ps_yst", bufs=1, space="PSUM") as ps_yst, \
             tc.tile_pool(name="ps_yb", bufs=1, space="PSUM") as ps_yb:
            ps_sum = ps_yst.tile([1, P], F32, name="ps_sum2", tag="ps_sum2")
            ps_ssq = ps_yst.tile([1, P], F32, name="ps_ssq2", tag="ps_ssq2")
            for k in range(KX):
                nc.tensor.matmul(ps_sum, _r(ones_col), y1T[:, k, psl],
                                 start=(k == 0), stop=(k == KX - 1))
                sqy = pool_fsq.tile([P, P], F32R, name="sqy", tag="sqy")
                nc.scalar.square(sqy, y1T[:, k, psl])
                nc.tensor.matmul(ps_ssq, _r(ones_col), _r(sqy),
                                 start=(k == 0), stop=(k == KX - 1))
            e2 = pool_frow.tile([1, P], F32, name="e2y", tag="e2y")
            m2 = pool_frow.tile([1, P], F32, name="m2y", tag="m2y")
            inv = pool_frow.tile([1, P], F32, name="invy", tag="invy")
            nc.vector.tensor_scalar_mul(m_row[0:1, psl], ps_sum, 1.0 / D)
            nc.vector.tensor_scalar_mul(e2, ps_ssq, 1.0 / D)
            nc.vector.tensor_mul(m2, _f(m_row[0:1, psl]), _f(m_row[0:1, psl]))
            nc.vector.tensor_sub(e2, e2, m2)
            nc.scalar.activation(inv, e2, AF.Sqrt, bias=eps_t[0:1])
            nc.vector.reciprocal_approx_fast(r_row[0:1, psl], inv)
            nc.vector.tensor_copy(r_rowR[0:1, psl], r_row[0:1, psl])
            bcm = ps_yb.tile([P, P], F32, name="bcm", tag="bcm")
            nc.tensor.matmul(bcm, _r(ones_row), m_row[0:1, psl])
            nc.vector.tensor_copy(myB[:, psl], bcm)
            bcr = ps_yb.tile([P, P], F32, name="bcr", tag="bcr")
            nc.tensor.matmul(bcr, _r(ones_row), r_rowR[0:1, psl])
            nc.vector.tensor_copy(ryB[:, psl], bcr)
        for k in range(KX):
            tmp = pool_ftmp.tile([P, P], F32, name="tmpn", tag="tmpn")
            nc.vector.tensor_sub(tmp, y1T[:, k, psl], myB[:, psl])
            nc.vector.tensor_mul(y1n[:, k, psl], tmp, ryB[:, psl])

    def ff1_piece(piece, ps_f, y2a=None):
        """ff1 + gelu for one piece; optionally interleave ff2 pass 1."""
        psl = slice(piece * P, (piece + 1) * P)
        for j in range(FFP):
            ps = ps_f.tile([P, P], F32, name="ps_f_t", tag="ps_f_t")
            for k in range(KX):
                nc.tensor.matmul(ps, w1sb[:, j, k, :], y1n[:, k, psl],
                                 start=(k == 0), stop=(k == KX - 1))
            nc.scalar.activation(g_sb[:, j, psl], ps, AF.Gelu,
                                 bias=b1_col[:, j:j + 1])
            if y2a is not None:
                for mi in range(4):
                    nc.tensor.matmul(y2a[mi], w2sb[:, j, mi * P:(mi + 1) * P],
                                     g_sb[:, j, :],
                                     start=(j == 0), stop=(j == FFP - 1))

    with tc.tile_pool(name="ps_f", bufs=2, space="PSUM") as ps_f:

        def emit_y2(y2a, mi):
            yt = pool_yo.tile([P, QS], F32, name="yt", tag="yt")
            nc.vector.scalar_tensor_tensor(
                out=yt, in0=y2a[mi % 4], scalar=b2_col[:, mi:mi + 1],
                in1=y1T[:, mi, :], op0=ALU.add, op1=ALU.add)
            nc.sync.dma_start(d["out"][mi * P:(mi + 1) * P, :], yt)

        # piece A: runs while the chunk-B ReduceScatter is still in flight
        piece_prep(0)
        ff1_piece(0, ps_f)
        # piece B + ff2 pass 1 (needs both pieces' activations per block)
        piece_prep(1)
        ps_y2_cm = tc.tile_pool(name="ps_y2", bufs=1, space="PSUM")
        ps_y2 = ps_y2_cm.__enter__()
        y2a = [ps_y2.tile([P, QS], F32, name=f"y2a_{i}", tag=f"y2a_{i}")
               for i in range(4)]
        ff1_piece(1, ps_f, y2a)
        for mi in range(4):
            emit_y2(y2a, mi)
        # ff2 pass 2 over the resident activations
        y2a = [ps_y2.tile([P, QS], F32, name=f"y2a_{i}", tag=f"y2a_{i}")
               for i in range(4)]
        for j in range(FFP):
            for mi in range(4, KX):
                nc.tensor.matmul(y2a[mi - 4], w2sb[:, j, mi * P:(mi + 1) * P],
                                 g_sb[:, j, :],
                                 start=(j == 0), stop=(j == FFP - 1))
        for mi in range(4, KX):
            emit_y2(y2a, mi)
        ps_y2_cm.__exit__(None, None, None)

    pool_yo.close()
    pool_ftmp.close()
    pool_frow.close()
    pool_fsq.close()
    pool_rsld.close()
    pool_f.close()
    pool_w1.close()
    const.close()
    dram.close()


def host_prep(inputs):
    """Fold layernorm gains/biases into (bf16) weights; build per-core shards."""
    f32 = np.float32
    bf = ml_dtypes.bfloat16
    x = np.asarray(inputs["x"], f32)
    z = np.asarray(inputs["z"], f32)
    mask = np.asarray(inputs["attn_mask"], f32)
    gq = np.asarray(inputs["gq"], np.float64)
    bq = np.asarray(inputs["bq"], np.float64)
    gkv = np.asarray(inputs["gkv"], np.float64)
    bkv = np.asarray(inputs["bkv"], np.float64)
    gff = np.asarray(inputs["gff"], np.float64)
    bff = np.asarray(inputs["bff"], np.float64)
    ipw = np.asarray(inputs["in_proj_w"], np.float64)
    ipb = np.asarray(inputs["in_proj_b"], np.float64)
    out_w = np.asarray(inputs["out_w"], f32)
    out_b = np.asarray(inputs["out_b"], f32)
    w1 = np.asarray(inputs["w1"], np.float64)
    b1 = np.asarray(inputs["b1"], np.float64)
    w2 = np.asarray(inputs["w2"], f32)
    b2 = np.asarray(inputs["b2"], f32)

    wq, wk, wv = ipw[:D], ipw[D:2 * D], ipw[2 * D:]
    pq, pk, pv = ipb[:D], ipb[D:2 * D], ipb[2 * D:]
    scale = 1.0 / np.sqrt(HD)
    wq2 = ((wq * gq[None, :]) * scale).astype(bf)
    pq2 = ((wq @ bq + pq) * scale).astype(f32)
    wk2 = (wk * gkv[None, :]).astype(bf)
    pk2 = (wk @ bkv + pk).astype(f32)
    wv2 = (wv * gkv[None, :]).astype(bf)
    pv2 = (wv @ bkv + pv).astype(f32)
    # rowsums of the *rounded* weights so the mean correction is consistent
    wq2r = wq2.astype(np.float64)
    wk2r = wk2.astype(np.float64)
    wv2r = wv2.astype(np.float64)

    w1b = (w1 * gff[None, :]).astype(bf)
    b12 = (w1b.astype(np.float64) @ bff + b1).astype(f32)
    b1c = np.ascontiguousarray(b12.reshape(FFP, P).T)
    w1T = np.ascontiguousarray(w1b.T)                              # (D, FF)
    w1p = np.ascontiguousarray(
        w1T.reshape(KX, P, FFP, P).transpose(2, 1, 0, 3))
    w2T = np.ascontiguousarray(w2.T.astype(bf))                    # (FF, D)

    def pack_kxf(wT):  # (D, FC) bf16 -> (P, D//P, FC)
        return np.ascontiguousarray(wT.reshape(KX, P, FC).transpose(1, 0, 2))

    cb = np.concatenate(
        [np.eye(P, dtype=f32), np.ones((P, 64), f32)], axis=1).astype(bf)

    in_maps = []
    for c in range(NCORES):
        b, hg = c // 4, c % 4
        fs = slice(FC * hg, FC * hg + FC)
        qidx = np.r_[P * hg:P * hg + P, 512 + P * hg:512 + P * hg + P]
        xTb = np.ascontiguousarray(x[b].T)                         # (D, Q)
        mk = mask[16 * b + NH * hg:16 * b + NH * hg + NH]          # (NH, Q, S)
        mkT = mk.transpose(0, 2, 1)                                # (NH, S, Q)
        m5 = mkT.reshape(2, 2, S, Q).transpose(0, 2, 1, 3)         # (pair,S,hh,Q)
        maskT = np.ascontiguousarray(
            np.stack([m5[:, :, :, 0:QC], m5[:, :, :, QC:]],
                     axis=0)).astype(bf)                           # (ci,pair,S,hh,QC)
        in_maps.append({
            "ones_t": np.ones((P, P), f32),
            "cb": cb,
            "xT": xTb.astype(bf),
            "zT": np.ascontiguousarray(z[b].T).astype(bf),
            "xq": np.ascontiguousarray(xTb[:, qidx]),
            "maskT": maskT,
            "wqT": pack_kxf(np.ascontiguousarray(wq2[fs].T)),
            "wkT": pack_kxf(np.ascontiguousarray(wk2[fs].T)),
            "wvT": pack_kxf(np.ascontiguousarray(wv2[fs].T)),
            "adjq": np.ascontiguousarray(
                np.stack([-wq2r[fs].sum(1), pq2[fs]]).astype(f32)),
            "adjk": np.ascontiguousarray(
                np.stack([-wk2r[fs].sum(1), pk2[fs]]).astype(f32)),
            "adjv": np.ascontiguousarray(
                np.stack([-wv2r[fs].sum(1), pv2[fs]]).astype(f32)),
            "owp": np.ascontiguousarray(
                out_w[:, fs].T.reshape(2, P, D).transpose(1, 0, 2)).astype(bf),
            "outb": out_b,
            "b1c": b1c,
            "b2": b2,
            "w1p": w1p,
            "w2T": w2T,
        })
    return in_maps


_NC_CACHE = None


def kernel(**inputs) -> np.ndarray:
    global _NC_CACHE, LAST_RESULT
    from concourse.bass_utils import run_bass_kernel_spmd

    in_maps = host_prep(inputs)
    if _NC_CACHE is None:
        _NC_CACHE = build_nc()
    res = run_bass_kernel_spmd(
        _NC_CACHE, in_maps, core_ids=list(range(NCORES)),
        trace=bool(os.environ.get("BASS_TRACE")),
    )
    LAST_RESULT = res
    out = np.empty((B, Q, D), np.float32)
    for c in range(NCORES):
        b, hg = c // 4, c % 4
        yT = res.results[c]["out"]                    # (D, QS)
        out[b, P * hg:P * hg + P, :] = yT[:, 0:P].T
        out[b, 512 + P * hg:512 + P * hg + P, :] = yT[:, P:2 * P].T
    return out


# revision 17
# speedup vs baseline: 1.0192x; 1.0192x over previous
# Bass/Tile TRN2 kernel for nn_BiasedCrossDecoderLayer (dense cross-attention
# transformer decoder layer), SPMD over 8 NeuronCores.
#
# Sharding: core c -> batch b = c//4, head-group hg = c%4 (4 of 16 heads).
# Attention is head-parallel and processed in two query chunks of 512; after
# each chunk the out-projection partial sums are ReduceScattered (bf16) within
# the 4-core batch group so the collective overlaps the next chunk's compute.
# Query ownership is interleaved: core c owns queries [128c,128c+128) of chunk
# A and [512+128c, 512+128c+128) of chunk B (the host gather restores order),
# so each chunk's RS delivers a 128-query piece directly.  The FFN then runs
# sequence-parallel on the core's 256 owned queries with the full 4096 hidden.
#
# LayerNorms are folded into the weights host-side (bf16 weights; rank-2
# [mean;std] correction matmuls in fp32r appended to each PSUM group).  The
# attention mask is added to the logits by an identity-matmul into the same
# PSUM accumulation group (no DVE pass); softmax denominators come from a
# ones-column appended to V (M=65 PV matmul).  The whole heavy datapath is
# bf16 (weights, activations, mask, probs) which halves DMA traffic and
# enables fast weight loads; PSUM accumulation stays fp32.  QK logits for a
# head pair run concurrently in two 64-row PE groups (K=64 row tiling).
#
# The FFN weights (w1 AND w2, bf16) are DMA'd into SBUF during attention so
# ff1/ff2 run back-to-back per hidden block with zero weight stalls.

import os
import sys

import numpy as np

sys.path.insert(0, "/opt/trn_rl_repo")

import ml_dtypes  # noqa: E402

import concourse.bass as bass  # noqa: E402
import concourse.mybir as mybir  # noqa: E402
import concourse.tile as tile  # noqa: E402
from concourse import bacc  # noqa: E402

F32 = mybir.dt.float32
F32R = mybir.dt.float32r
BF16 = mybir.dt.bfloat16
AF = mybir.ActivationFunctionType
ALU = mybir.AluOpType

B, Q, S, D, H = 2, 1024, 2048, 1024, 16
HD = D // H       # 64
FF = 4 * D
EPS = 1e-5
NCORES = 8
NH = 4            # heads per core
FC = NH * HD      # 256 qkv feature dims per core
QS = 256          # queries owned per core (two 128-query pieces)
QC = 512          # attention query chunk
P = 128
KX = D // P       # 8 k-tiles over the model dim
FFP = FF // P     # 32 hidden blocks
NST = S // P      # 16 s-tiles

REPLICA_GROUPS = [[0, 1, 2, 3], [4, 5, 6, 7]]

LAST_RESULT = None  # BassKernelResults of the most recent run (for test.py)


def _r(ap):
    return ap.bitcast(F32R)


def _f(ap):
    return ap.bitcast(F32)


def build_nc():
    nc = bacc.Bacc(
        "TRN2",
        target_bir_lowering=False,
        debug=False,
        num_devices=NCORES,
        name="biased_cross_decoder",
    )

    d = {}
    d["ones_t"] = nc.dram_tensor("ones_t", [P, P], F32R, kind="ExternalInput").ap()
    d["cb"] = nc.dram_tensor("cb", [P, P + 64], BF16, kind="ExternalInput").ap()
    d["xT"] = nc.dram_tensor("xT", [D, Q], BF16, kind="ExternalInput").ap()
    d["zT"] = nc.dram_tensor("zT", [D, S], BF16, kind="ExternalInput").ap()
    d["xq"] = nc.dram_tensor("xq", [D, QS], F32, kind="ExternalInput").ap()
    d["maskT"] = nc.dram_tensor("maskT", [2, 2, S, 2, QC], BF16,
                                kind="ExternalInput").ap()
    d["wqT"] = nc.dram_tensor("wqT", [P, KX, FC], BF16, kind="ExternalInput").ap()
    d["wkT"] = nc.dram_tensor("wkT", [P, KX, FC], BF16, kind="ExternalInput").ap()
    d["wvT"] = nc.dram_tensor("wvT", [P, KX, FC], BF16, kind="ExternalInput").ap()
    d["adjq"] = nc.dram_tensor("adjq", [2, FC], F32R, kind="ExternalInput").ap()
    d["adjk"] = nc.dram_tensor("adjk", [2, FC], F32R, kind="ExternalInput").ap()
    d["adjv"] = nc.dram_tensor("adjv", [2, FC], F32R, kind="ExternalInput").ap()
    d["owp"] = nc.dram_tensor("owp", [P, 2, D], BF16, kind="ExternalInput").ap()
    d["outb"] = nc.dram_tensor("outb", [D], F32, kind="ExternalInput").ap()
    d["b1c"] = nc.dram_tensor("b1c", [P, FFP], F32, kind="ExternalInput").ap()
    d["b2"] = nc.dram_tensor("b2", [D], F32, kind="ExternalInput").ap()
    d["w1p"] = nc.dram_tensor("w1p", [FFP, P, KX, P], BF16,
                              kind="ExternalInput").ap()
    d["w2T"] = nc.dram_tensor("w2T", [FF, D], BF16, kind="ExternalInput").ap()
    d["out"] = nc.dram_tensor("out", [D, QS], F32, kind="ExternalOutput").ap()

    with tile.TileContext(nc) as tc:
        build_tile_program(tc, nc, d)
    nc.compile()
    return nc


class _Pool:
    """Keeps the tile_pool context manager alive; allows explicit close."""

    def __init__(self, cm):
        self._cm = cm
        self.pool = cm.__enter__()

    def tile(self, *a, **kw):
        kw.setdefault("name", kw.get("tag") or "t")
        return self.pool.tile(*a, **kw)

    def close(self):
        self._cm.__exit__(None, None, None)


def build_tile_program(tc, nc, d):
    # ---------------- persistent constants ----------------
    const = _Pool(tc.tile_pool(name="const", bufs=1))
    dram = _Pool(tc.tile_pool(name="dram", bufs=1, space="DRAM"))

    ones_sb = const.tile([P, P], F32R, tag="ones_sb")
    nc.sync.dma_start(ones_sb, d["ones_t"])
    cb_sb = const.tile([P, P + 64], BF16, tag="cb_sb")
    nc.sync.dma_start(cb_sb, d["cb"])
    ident = cb_sb[:, 0:P]            # bf16 identity (mask-add matmul lhsT)
    ones_bcol = cb_sb[:, P:P + 1]    # bf16 ones column (bf16 stat sums)
    ones_col = ones_sb[:, 0:1]       # f32r ones column (fp32 stat sums)
    ones_row = ones_sb[0:1, :]       # f32r row (partition broadcasts)

    eps_t = const.tile([1, 1], F32, tag="eps")
    nc.vector.memset(eps_t, EPS)
    outb_col = const.tile([P, KX], F32, tag="outb_col")
    nc.sync.dma_start(outb_col, d["outb"].rearrange("(o p) -> p o", p=P))
    b2_col = const.tile([P, KX], F32, tag="b2_col")
    nc.sync.dma_start(b2_col, d["b2"].rearrange("(o p) -> p o", p=P))
    b1_col = const.tile([P, FFP], F32, tag="b1_col")
    nc.sync.dma_start(b1_col, d["b1c"])
    rz_col = const.tile([P, NST], F32R, tag="rz_col")

    rs_scr = dram.tile([1, S], F32R, tag="rs_scr")
    rs_in = [dram.tile([4, D, P], BF16, name=f"rs_in{i}", tag=f"rs_in{i}")
             for i in range(2)]
    rs_out = [dram.tile([D, P], BF16, name=f"rs_out{i}", tag=f"rs_out{i}")
              for i in range(2)]

    # ---------------- long-lived right-side pools ----------------
    pool_qkv = _Pool(tc.tile_pool(name="qkv", bufs=1, side="right"))
    qT = pool_qkv.tile([P, 2, Q], BF16, tag="qT")
    kT = pool_qkv.tile([P, 2, S], BF16, tag="kT")
    v_sb = pool_qkv.tile([P, NST, NH, HD + 1], BF16, tag="v_sb")
    ow_sb = pool_qkv.tile([P, 2, D], BF16, tag="ow_sb")
    nc.sync.dma_start(ow_sb, d["owp"])

    # ---------------- phase A scratch (left stack) ----------------
    pool_x = _Pool(tc.tile_pool(name="px", bufs=1))
    pool_z = _Pool(tc.tile_pool(name="pz", bufs=1))
    pool_w = _Pool(tc.tile_pool(name="pw", bufs=1))
    pool_adj = _Pool(tc.tile_pool(name="adj", bufs=1))
    pool_bc = _Pool(tc.tile_pool(name="bc", bufs=1))
    pool_sq = _Pool(tc.tile_pool(name="sq", bufs=3))
    pool_rows = _Pool(tc.tile_pool(name="rows", bufs=2))

    xT = pool_x.tile([P, KX, Q], BF16, tag="xT")
    for k in range(KX):
        nc.sync.dma_start(xT[:, k, :], d["xT"][k * P:(k + 1) * P, :])
    zT = pool_z.tile([P, KX, S], BF16, tag="zT")
    for h2 in range(2):
        for k in range(KX):
            nc.scalar.dma_start(zT[:, k, h2 * 1024:(h2 + 1) * 1024],
                                d["zT"][k * P:(k + 1) * P, h2 * 1024:(h2 + 1) * 1024])
    wq_sb = pool_w.tile([P, KX, FC], BF16, tag="wq_sb")
    nc.sync.dma_start(wq_sb, d["wqT"])
    wk_sb = pool_w.tile([P, KX, FC], BF16, tag="wk_sb")
    nc.sync.dma_start(wk_sb, d["wkT"])
    wv_sb = pool_w.tile([P, KX, FC], BF16, tag="wv_sb")
    nc.sync.dma_start(wv_sb, d["wvT"])
    adjq_w = pool_w.tile([2, FC], F32R, tag="adjq_w")
    nc.sync.dma_start(adjq_w, d["adjq"])
    adjk_w = pool_w.tile([2, FC], F32R, tag="adjk_w")
    nc.sync.dma_start(adjk_w, d["adjk"])
    adjv_w = pool_w.tile([2, FC], F32R, tag="adjv_w")
    nc.sync.dma_start(adjv_w, d["adjv"])

    adjx = pool_adj.tile([2, Q], F32R, tag="adjx")      # [mean ; std] rows
    adjz = pool_adj.tile([2, S], F32R, tag="adjz")
    rxB = pool_bc.tile([P, Q], F32, tag="rxB")          # 1/std broadcast
    rzB = pool_bc.tile([P, S], F32, tag="rzB")

    def ln_stats(aT, T, adj, rB, ps_stats, scr=None):
        """Per 512-token chunk: LN stats -> adj=[mean;std] rows and a
        [P, T] broadcast of 1/std (via gpsimd partition_broadcast)."""
        for ch in range(T // 512):
            sl = slice(ch * 512, (ch + 1) * 512)
            ps_sum = ps_stats.tile([1, 512], F32, name="ps_sum", tag="ps_sum")
            ps_ssq = ps_stats.tile([1, 512], F32, name="ps_ssq", tag="ps_ssq")
            for k in range(KX):
                nc.tensor.matmul(ps_sum, ones_bcol, aT[:, k, sl],
                                 start=(k == 0), stop=(k == KX - 1))
                sq = pool_sq.tile([P, 512], BF16, name="sq", tag="sq")
                nc.scalar.square(sq, aT[:, k, sl])
                nc.tensor.matmul(ps_ssq, ones_bcol, sq,
                                 start=(k == 0), stop=(k == KX - 1))
            e2 = pool_rows.tile([1, 512], F32, name="e2", tag="e2")
            m2 = pool_rows.tile([1, 512], F32, name="m2", tag="m2")
            inv = pool_rows.tile([1, 512], F32R, name="inv", tag="inv")
            rr = pool_rows.tile([1, 512], F32R, name="rr", tag="rr")
            nc.vector.tensor_scalar_mul(adj[0:1, sl], ps_sum, 1.0 / D)  # mean
            nc.vector.tensor_scalar_mul(e2, ps_ssq, 1.0 / D)            # E[x^2]
            nc.vector.tensor_mul(m2, adj[0:1, sl], adj[0:1, sl])
            nc.vector.tensor_sub(e2, e2, m2)                            # var
            nc.scalar.activation(inv, e2, AF.Sqrt, bias=eps_t[0:1])     # std
            nc.vector.reciprocal_approx_fast(_f(rr), _f(inv))
            nc.scalar.dma_start(adj[1:2, sl], inv)   # cross-partition row move
            nc.gpsimd.partition_broadcast(rB[:, sl], _f(rr))
            if scr is not None:
                nc.scalar.dma_start(scr[0:1, sl], rr)

    # ---- x statistics + q projection ----
    with tc.tile_pool(name="ps_sx", bufs=2, space="PSUM") as ps_sx:
        ln_stats(xT, Q, adjx, rxB, ps_sx)

    with tc.tile_pool(name="ps_q", bufs=3, space="PSUM") as ps_qk:
        for m in range(2):
            for ch in range(2):
                sl = slice(ch * 512, (ch + 1) * 512)
                ps = ps_qk.tile([P, 512], F32, name="ps_qk_t", tag="ps_qk_t")
                for k in range(KX):
                    nc.tensor.matmul(ps, wq_sb[:, k, m * P:(m + 1) * P],
                                     xT[:, k, sl], start=(k == 0), stop=False)
                nc.tensor.matmul(ps, adjq_w[:, m * P:(m + 1) * P],
                                 _r(adjx[:, sl]), start=False, stop=True)
                nc.vector.tensor_mul(qT[:, m, sl], ps, rxB[:, sl])

    # ---- z statistics + k/v projections ----
    with tc.tile_pool(name="ps_sz", bufs=2, space="PSUM") as ps_sz:
        ln_stats(zT, S, adjz, rzB, ps_sz, scr=rs_scr)
    nc.scalar.dma_start(rz_col, rs_scr.rearrange("a (i p) -> (a p) i", p=P))

    # softmax-denominator ones column
    nc.sync.dma_start(
        v_sb[:, :, :, HD:HD + 1],
        d["cb"][:, P:P + 64].rearrange("p (a b c) -> p a b c", a=NST, c=1))

    with tc.tile_pool(name="ps_k", bufs=3, space="PSUM") as ps_qk, \
         tc.tile_pool(name="ps_v", bufs=2, space="PSUM") as ps_v:
        for m in range(2):
            for ch in range(4):
                sl = slice(ch * 512, (ch + 1) * 512)
                ps = ps_qk.tile([P, 512], F32, name="ps_qk_t", tag="ps_qk_t")
                for k in range(KX):
                    nc.tensor.matmul(ps, wk_sb[:, k, m * P:(m + 1) * P],
                                     zT[:, k, sl], start=(k == 0), stop=False)
                nc.tensor.matmul(ps, adjk_w[:, m * P:(m + 1) * P],
                                 _r(adjz[:, sl]), start=False, stop=True)
                nc.vector.tensor_mul(kT[:, m, sl], ps, rzB[:, sl])

        for t in range(NST):
            ps = ps_v.tile([P, FC], F32, name="ps_v_t", tag="ps_v_t")
            for k in range(KX):
                nc.tensor.matmul(ps, zT[:, k, t * P:(t + 1) * P],
                                 wv_sb[:, k, :], start=(k == 0), stop=False)
            nc.tensor.matmul(ps, _r(adjz[:, t * P:(t + 1) * P]), _r(adjv_w),
                             start=False, stop=True)
            nc.vector.tensor_scalar_mul(
                v_sb[:, t, :, 0:HD],
                ps.rearrange("p (h e) -> p h e", h=NH),
                _f(rz_col[:, t:t + 1]))

    pool_rows.close()
    pool_sq.close()
    pool_bc.close()
    pool_adj.close()
    pool_w.close()
    pool_z.close()
    pool_x.close()

    # ---------------- resident FFN weights (prefetched during attention) ----
    pool_w1 = _Pool(tc.tile_pool(name="w1r", bufs=1))
    w1sb = pool_w1.tile([P, FFP, KX, P], BF16, tag="w1sb")
    w2sb = pool_w1.tile([P, FFP, D], BF16, tag="w2sb")
    for j0 in range(0, FFP, 8):
        nc.scalar.dma_start(
            w1sb[:, j0:j0 + 8],
            d["w1p"][j0:j0 + 8].rearrange("j p k o -> p j k o"))
        nc.scalar.dma_start(
            w2sb[:, j0:j0 + 8],
            d["w2T"][j0 * P:(j0 + 8) * P, :].rearrange("(j p) o -> p j o", p=P))

    # =================== attention (query-chunked) ===================
    pool_att2 = _Pool(tc.tile_pool(name="att2", bufs=1))
    pool_mk = _Pool(tc.tile_pool(name="mk", bufs=9))
    pool_pr0 = _Pool(tc.tile_pool(name="pr0", bufs=3))
    pool_pr1 = _Pool(tc.tile_pool(name="pr1", bufs=3))
    pool_nrm = _Pool(tc.tile_pool(name="nrm", bufs=2))
    pool_osb = _Pool(tc.tile_pool(name="osb", bufs=3))

    for ci in range(2):
        qsl = slice(ci * QC, (ci + 1) * QC)
        att2 = [pool_att2.tile([P, QC], BF16, name=f"att2_{p}", tag=f"att2_{p}")
                for p in range(2)]

        ps_lg0_cm = tc.tile_pool(name="ps_lg0", bufs=3, space="PSUM")
        ps_lg1_cm = tc.tile_pool(name="ps_lg1", bufs=3, space="PSUM")
        ps_att_cm = tc.tile_pool(name="ps_att", bufs=1, space="PSUM")
        ps_lg0 = ps_lg0_cm.__enter__()
        ps_lg1 = ps_lg1_cm.__enter__()
        ps_att = ps_att_cm.__enter__()

        for pair in range(2):
            attps = [ps_att.tile([HD + 1, QC], F32, name=f"attps{hh}",
                                 tag=f"attps{hh}") for hh in range(2)]

            def emit_pv(st, p0, p1):
                nc.tensor.matmul(attps[0], v_sb[:, st, 2 * pair, :], p0,
                                 start=(st == 0), stop=(st == NST - 1))
                nc.tensor.matmul(attps[1], v_sb[:, st, 2 * pair + 1, :], p1,
                                 start=(st == 0), stop=(st == NST - 1))

            prev = None
            for st in range(NST):
                ssl = slice(st * P, (st + 1) * P)
                mk = pool_mk.tile([P, 2, QC], BF16, name="mk", tag="mk")
                if st % 2 == 0:
                    nc.sync.dma_start(mk, d["maskT"][ci, pair, ssl, :, :])
                else:
                    nc.gpsimd.dma_start(mk, d["maskT"][ci, pair, ssl, :, :])
                lg0 = ps_lg0.tile([P, QC], F32, name="lg0", tag="lg0")
                lg1 = ps_lg1.tile([P, QC], F32, name="lg1", tag="lg1")
                # head-pair QK in two concurrent 64-row PE groups
                nc.tensor.matmul(lg0, kT[0:HD, pair, ssl], qT[0:HD, pair, qsl],
                                 start=True, stop=False)
                nc.tensor.matmul(lg1, kT[HD:P, pair, ssl], qT[HD:P, pair, qsl],
                                 start=True, stop=False)
                # mask add via identity matmul into the same PSUM group
                nc.tensor.matmul(lg0, ident, mk[:, 0, :], start=False, stop=True)
                nc.tensor.matmul(lg1, ident, mk[:, 1, :], start=False, stop=True)
                pr0 = pool_pr0.tile([P, QC], BF16, name="pr0", tag="pr0")
                nc.scalar.activation(pr0, lg0, AF.Exp)
                pr1 = pool_pr1.tile([P, QC], BF16, name="pr1", tag="pr1")
                nc.scalar.activation(pr1, lg1, AF.Exp)
                if prev is not None:
                    emit_pv(*prev)
                prev = (st, pr0, pr1)
            emit_pv(*prev)

            # normalize: att2[pair][64h:64h+64] = attps[h][0:64] / attps[h][64]
            for hh in range(2):
                den = pool_nrm.tile([1, QC], F32, name="den", tag="den")
                nc.vector.tensor_copy(den, attps[hh][HD:HD + 1, :])
                r0 = pool_nrm.tile([1, QC], F32, name="r0", tag="r0")
                nc.vector.reciprocal_approx_fast(r0, den)
                nbc = pool_nrm.tile([HD, QC], F32, name="nbc", tag="nbc")
                nc.gpsimd.partition_broadcast(nbc, r0)
                nc.vector.tensor_mul(att2[pair][HD * hh:HD * hh + HD, :],
                                     attps[hh][0:HD, :], nbc)

        ps_att_cm.__exit__(None, None, None)
        ps_lg1_cm.__exit__(None, None, None)
        ps_lg0_cm.__exit__(None, None, None)

        # ---- out-projection for this chunk + ReduceScatter ----
        with tc.tile_pool(name="ps_o", bufs=2, space="PSUM") as ps_o:
            for m in range(KX):
                ps = ps_o.tile([P, QC], F32, name="ps_o_t", tag="ps_o_t")
                nc.tensor.matmul(ps, ow_sb[:, 0, m * P:(m + 1) * P], att2[0],
                                 start=True, stop=False)
                nc.tensor.matmul(ps, ow_sb[:, 1, m * P:(m + 1) * P], att2[1],
                                 start=False, stop=True)
                ot = pool_osb.tile([P, QC], BF16, name="ot", tag="ot")
                nc.vector.tensor_copy(ot, ps)
                nc.scalar.dma_start(
                    rs_in[ci][:, m * P:(m + 1) * P, :].rearrange(
                        "r p q -> p r q"),
                    ot.rearrange("p (r q) -> p r q", r=4))
        nc.gpsimd.collective_compute(
            "ReduceScatter",
            ALU.add,
            replica_groups=REPLICA_GROUPS,
            ins=[rs_in[ci].opt()],
            outs=[rs_out[ci].opt()],
        )

    pool_osb.close()
    pool_nrm.close()
    pool_pr1.close()
    pool_pr0.close()
    pool_mk.close()
    pool_att2.close()
    pool_qkv.close()

    # =================== residual + FFN (sequence-parallel) ===================
    pool_f = _Pool(tc.tile_pool(name="ffn", bufs=1, side="right"))
    pool_rsld = _Pool(tc.tile_pool(name="rsld", bufs=3))
    pool_fsq = _Pool(tc.tile_pool(name="fsq", bufs=2))
    pool_frow = _Pool(tc.tile_pool(name="frow", bufs=2))
    pool_ftmp = _Pool(tc.tile_pool(name="ftmp", bufs=2))
    pool_yo = _Pool(tc.tile_pool(name="yout", bufs=3))

    y1T = pool_f.tile([P, KX, QS], F32R, tag="y1T")
    y1n = pool_f.tile([P, KX, QS], BF16, tag="y1n")
    g_sb = pool_f.tile([P, FFP, QS], BF16, tag="g_sb")
    m_row = pool_f.tile([1, QS], F32R, tag="m_row")
    r_row = pool_f.tile([1, QS], F32, tag="r_row")
    r_rowR = pool_f.tile([1, QS], F32R, tag="r_rowR")
    myB = pool_f.tile([P, QS], F32, tag="myB")
    ryB = pool_f.tile([P, QS], F32, tag="ryB")
    xq_sb = pool_f.tile([P, KX, QS], F32, tag="xq_sb")
    nc.gpsimd.dma_start(xq_sb, d["xq"].rearrange("(k p) q -> p k q", p=P))

    def piece_prep(piece):
        """y1 assembly + LN stats + normalize for one 128-query piece."""
        psl = slice(piece * P, (piece + 1) * P)
        for k in range(KX):
            rst = pool_rsld.tile([P, P], BF16, name="rst", tag="rst")
            nc.gpsimd.dma_start(rst, rs_out[piece][k * P:(k + 1) * P, :])
            nc.vector.scalar_tensor_tensor(
                out=y1T[:, k, psl], in0=rst, scalar=outb_col[:, k:k + 1],
                in1=xq_sb[:, k, psl], op0=ALU.add, op1=ALU.add)
        with tc.tile_pool(name="ps_yst", bufs=1, space="PSUM") as ps_yst, \
             tc.tile_pool(name="ps_yb", bufs=1, space="PSUM") as ps_yb:
            ps_sum = ps_yst.tile([1, P], F32, name="ps_sum2", tag="ps_sum2")
            ps_ssq = ps_yst.tile([1, P], F32, name="ps_ssq2", tag="ps_ssq2")
            for k in range(KX):
                nc.tensor.matmul(ps_sum, _r(ones_col), y1T[:, k, psl],
                                 start=(k == 0), stop=(k == KX - 1))
                sqy = pool_fsq.tile([P, P], F32R, name="sqy", tag="sqy")
                nc.scalar.square(sqy, y1T[:, k, psl])
                nc.tensor.matmul(ps_ssq, _r(ones_col), _r(sqy),
                                 start=(k == 0), stop=(k == KX - 1))
            e2 = pool_frow.tile([1, P], F32, name="e2y", tag="e2y")
            m2 = pool_frow.tile([1, P], F32, name="m2y", tag="m2y")
            inv = pool_frow.tile([1, P], F32, name="invy", tag="invy")
            nc.vector.tensor_scalar_mul(m_row[0:1, psl], ps_sum, 1.0 / D)
            nc.vector.tensor_scalar_mul(e2, ps_ssq, 1.0 / D)
            nc.vector.tensor_mul(m2, _f(m_row[0:1, psl]), _f(m_row[0:1, psl]))
            nc.vector.tensor_sub(e2, e2, m2)
            nc.scalar.activation(inv, e2, AF.Sqrt, bias=eps_t[0:1])
            nc.vector.reciprocal_approx_fast(r_row[0:1, psl], inv)
            nc.vector.tensor_copy(r_rowR[0:1, psl], r_row[0:1, psl])
            bcm = ps_yb.tile([P, P], F32, name="bcm", tag="bcm")
            nc.tensor.matmul(bcm, _r(ones_row), m_row[0:1, psl])
            nc.vector.tensor_copy(myB[:, psl], bcm)
            bcr = ps_yb.tile([P, P], F32, name="bcr", tag="bcr")
            nc.tensor.matmul(bcr, _r(ones_row), r_rowR[0:1, psl])
            nc.vector.tensor_copy(ryB[:, psl], bcr)
        for k in range(KX):
            tmp = pool_ftmp.tile([P, P], F32, name="tmpn", tag="tmpn")
            nc.vector.tensor_sub(tmp, y1T[:, k, psl], myB[:, psl])
            nc.vector.tensor_mul(y1n[:, k, psl], tmp, ryB[:, psl])

    def ff1_piece(piece, ps_f, y2a=None):
        """ff1 + gelu for one piece; optionally interleave ff2 pass 1."""
        psl = slice(piece * P, (piece + 1) * P)
        for j in range(FFP):
            ps = ps_f.tile([P, P], F32, name="ps_f_t", tag="ps_f_t")
            for k in range(KX):
                nc.tensor.matmul(ps, w1sb[:, j, k, :], y1n[:, k, psl],
                                 start=(k == 0), stop=(k == KX - 1))
            nc.scalar.activation(g_sb[:, j, psl], ps, AF.Gelu,
                                 bias=b1_col[:, j:j + 1])
            if y2a is not None:
                for mi in range(4):
                    nc.tensor.matmul(y2a[mi], w2sb[:, j, mi * P:(mi + 1) * P],
                                     g_sb[:, j, :],
                                     start=(j == 0), stop=(j == FFP - 1))

    with tc.tile_pool(name="ps_f", bufs=2, space="PSUM") as ps_f:

        def emit_y2(y2a, mi):
            yt = pool_yo.tile([P, QS], F32, name="yt", tag="yt")
            nc.vector.scalar_tensor_tensor(
                out=yt, in0=y2a[mi % 4], scalar=b2_col[:, mi:mi + 1],
                in1=y1T[:, mi, :], op0=ALU.add, op1=ALU.add)
            nc.scalar.dma_start(d["out"][mi * P:(mi + 1) * P, :], yt)

        # piece A: runs while the chunk-B ReduceScatter is still in flight
        tc.no_sync_barrier()
        piece_prep(0)
        ff1_piece(0, ps_f)
        # piece B + ff2 pass 1 (needs both pieces' activations per block)
        tc.no_sync_barrier()
        piece_prep(1)
        ps_y2_cm = tc.tile_pool(name="ps_y2", bufs=1, space="PSUM")
        ps_y2 = ps_y2_cm.__enter__()
        y2a = [ps_y2.tile([P, QS], F32, name=f"y2a_{i}", tag=f"y2a_{i}")
               for i in range(4)]
        ff1_piece(1, ps_f, y2a)
        for mi in range(4):
            emit_y2(y2a, mi)
        # ff2 pass 2 over the resident activations
        y2a = [ps_y2.tile([P, QS], F32, name=f"y2a_{i}", tag=f"y2a_{i}")
               for i in range(4)]
        for j in range(FFP):
            for mi in range(4, KX):
                nc.tensor.matmul(y2a[mi - 4], w2sb[:, j, mi * P:(mi + 1) * P],
                                 g_sb[:, j, :],
                                 start=(j == 0), stop=(j == FFP - 1))
        for mi in range(4, KX):
            emit_y2(y2a, mi)
        ps_y2_cm.__exit__(None, None, None)

    pool_yo.close()
    pool_ftmp.close()
    pool_frow.close()
    pool_fsq.close()
    pool_rsld.close()
    pool_f.close()
    pool_w1.close()
    const.close()
    dram.close()


def host_prep(inputs):
    """Fold layernorm gains/biases into (bf16) weights; build per-core shards."""
    f32 = np.float32
    bf = ml_dtypes.bfloat16
    x = np.asarray(inputs["x"], f32)
    z = np.asarray(inputs["z"], f32)
    mask = np.asarray(inputs["attn_mask"], f32)
    gq = np.asarray(inputs["gq"], np.float64)
    bq = np.asarray(inputs["bq"], np.float64)
    gkv = np.asarray(inputs["gkv"], np.float64)
    bkv = np.asarray(inputs["bkv"], np.float64)
    gff = np.asarray(inputs["gff"], np.float64)
    bff = np.asarray(inputs["bff"], np.float64)
    ipw = np.asarray(inputs["in_proj_w"], np.float64)
    ipb = np.asarray(inputs["in_proj_b"], np.float64)
    out_w = np.asarray(inputs["out_w"], f32)
    out_b = np.asarray(inputs["out_b"], f32)
    w1 = np.asarray(inputs["w1"], np.float64)
    b1 = np.asarray(inputs["b1"], np.float64)
    w2 = np.asarray(inputs["w2"], f32)
    b2 = np.asarray(inputs["b2"], f32)

    wq, wk, wv = ipw[:D], ipw[D:2 * D], ipw[2 * D:]
    pq, pk, pv = ipb[:D], ipb[D:2 * D], ipb[2 * D:]
    scale = 1.0 / np.sqrt(HD)
    wq2 = ((wq * gq[None, :]) * scale).astype(bf)
    pq2 = ((wq @ bq + pq) * scale).astype(f32)
    wk2 = (wk * gkv[None, :]).astype(bf)
    pk2 = (wk @ bkv + pk).astype(f32)
    wv2 = (wv * gkv[None, :]).astype(bf)
    pv2 = (wv @ bkv + pv).astype(f32)
    # rowsums of the *rounded* weights so the mean correction is consistent
    wq2r = wq2.astype(np.float64)
    wk2r = wk2.astype(np.float64)
    wv2r = wv2.astype(np.float64)

    w1b = (w1 * gff[None, :]).astype(bf)
    b12 = (w1b.astype(np.float64) @ bff + b1).astype(f32)
    b1c = np.ascontiguousarray(b12.reshape(FFP, P).T)
    w1T = np.ascontiguousarray(w1b.T)                              # (D, FF)
    w1p = np.ascontiguousarray(
        w1T.reshape(KX, P, FFP, P).transpose(2, 1, 0, 3))
    w2T = np.ascontiguousarray(w2.T.astype(bf))                    # (FF, D)

    def pack_kxf(wT):  # (D, FC) bf16 -> (P, D//P, FC)
        return np.ascontiguousarray(wT.reshape(KX, P, FC).transpose(1, 0, 2))

    cb = np.concatenate(
        [np.eye(P, dtype=f32), np.ones((P, 64), f32)], axis=1).astype(bf)

    in_maps = []
    for c in range(NCORES):
        b, hg = c // 4, c % 4
        fs = slice(FC * hg, FC * hg + FC)
        qidx = np.r_[P * hg:P * hg + P, 512 + P * hg:512 + P * hg + P]
        xTb = np.ascontiguousarray(x[b].T)                         # (D, Q)
        mk = mask[16 * b + NH * hg:16 * b + NH * hg + NH]          # (NH, Q, S)
        mkT = mk.transpose(0, 2, 1)                                # (NH, S, Q)
        m5 = mkT.reshape(2, 2, S, Q).transpose(0, 2, 1, 3)         # (pair,S,hh,Q)
        maskT = np.ascontiguousarray(
            np.stack([m5[:, :, :, 0:QC], m5[:, :, :, QC:]],
                     axis=0)).astype(bf)                           # (ci,pair,S,hh,QC)
        in_maps.append({
            "ones_t": np.ones((P, P), f32),
            "cb": cb,
            "xT": xTb.astype(bf),
            "zT": np.ascontiguousarray(z[b].T).astype(bf),
            "xq": np.ascontiguousarray(xTb[:, qidx]),
            "maskT": maskT,
            "wqT": pack_kxf(np.ascontiguousarray(wq2[fs].T)),
            "wkT": pack_kxf(np.ascontiguousarray(wk2[fs].T)),
            "wvT": pack_kxf(np.ascontiguousarray(wv2[fs].T)),
            "adjq": np.ascontiguousarray(
                np.stack([-wq2r[fs].sum(1), pq2[fs]]).astype(f32)),
            "adjk": np.ascontiguousarray(
                np.stack([-wk2r[fs].sum(1), pk2[fs]]).astype(f32)),
            "adjv": np.ascontiguousarray(
                np.stack([-wv2r[fs].sum(1), pv2[fs]]).astype(f32)),
            "owp": np.ascontiguousarray(
                out_w[:, fs].T.reshape(2, P, D).transpose(1, 0, 2)).astype(bf),
            "outb": out_b,
            "b1c": b1c,
            "b2": b2,
            "w1p": w1p,
            "w2T": w2T,
        })
    return in_maps


_NC_CACHE = None


def kernel(**inputs) -> np.ndarray:
    global _NC_CACHE, LAST_RESULT
    from concourse.bass_utils import run_bass_kernel_spmd

    in_maps = host_prep(inputs)
    if _NC_CACHE is None:
        _NC_CACHE = build_nc()
    res = run_bass_kernel_spmd(
        _NC_CACHE, in_maps, core_ids=list(range(NCORES)),
        trace=bool(os.environ.get("BASS_TRACE")),
    )
    LAST_RESULT = res
    out = np.empty((B, Q, D), np.float32)
    for c in range(NCORES):
        b, hg = c // 4, c % 4
        yT = res.results[c]["out"]                    # (D, QS)
        out[b, P * hg:P * hg + P, :] = yT[:, 0:P].T
        out[b, 512 + P * hg:512 + P * hg + P, :] = yT[:, P:2 * P].T
    return out


# revision 18
# speedup vs baseline: 1.0535x; 1.0337x over previous
# Bass/Tile TRN2 kernel for nn_BiasedCrossDecoderLayer (dense cross-attention
# transformer decoder layer), SPMD over 8 NeuronCores.
#
# Sharding: core c -> batch b = c//4, head-group hg = c%4 (4 of 16 heads).
# Attention is head-parallel and processed in two query chunks of 512; after
# each chunk the out-projection partial sums are ReduceScattered (bf16) within
# the 4-core batch group so the collective overlaps the next chunk's compute.
# Query ownership is interleaved: core c owns queries [128c,128c+128) of chunk
# A and [512+128c, 512+128c+128) of chunk B (the host gather restores order),
# so each chunk's RS delivers a 128-query piece directly.  The FFN then runs
# sequence-parallel on the core's 256 owned queries with the full 4096 hidden.
#
# LayerNorms are folded into the weights host-side (bf16 weights; rank-2
# [mean;std] correction matmuls in fp32r appended to each PSUM group).  The
# attention mask is added to the logits by an identity-matmul into the same
# PSUM accumulation group (no DVE pass); softmax denominators come from a
# ones-column appended to V (M=65 PV matmul).  The whole heavy datapath is
# bf16 (weights, activations, mask, probs) which halves DMA traffic and
# enables fast weight loads; PSUM accumulation stays fp32.  QK logits for a
# head pair run concurrently in two 64-row PE groups (K=64 row tiling).
#
# The FFN weights (w1 AND w2, bf16) are DMA'd into SBUF during attention so
# ff1/ff2 run back-to-back per hidden block with zero weight stalls.

import os
import sys

import numpy as np

sys.path.insert(0, "/opt/trn_rl_repo")

import ml_dtypes  # noqa: E402

import concourse.bass as bass  # noqa: E402
import concourse.mybir as mybir  # noqa: E402
import concourse.tile as tile  # noqa: E402
from concourse import bacc  # noqa: E402

F32 = mybir.dt.float32
F32R = mybir.dt.float32r
BF16 = mybir.dt.bfloat16
AF = mybir.ActivationFunctionType
ALU = mybir.AluOpType

B, Q, S, D, H = 2, 1024, 2048, 1024, 16
HD = D // H       # 64
FF = 4 * D
EPS = 1e-5
NCORES = 8
NH = 4            # heads per core
FC = NH * HD      # 256 qkv feature dims per core
QS = 256          # queries owned per core (two 128-query pieces)
QC = 512          # attention query chunk
P = 128
KX = D // P       # 8 k-tiles over the model dim
FFP = FF // P     # 32 hidden blocks
NST = S // P      # 16 s-tiles

REPLICA_GROUPS = [[0, 1, 2, 3], [4, 5, 6, 7]]

LAST_RESULT = None  # BassKernelResults of the most recent run (for test.py)


def _r(ap):
    return ap.bitcast(F32R)


def _f(ap):
    return ap.bitcast(F32)


def build_nc():
    nc = bacc.Bacc(
        "TRN2",
        target_bir_lowering=False,
        debug=False,
        num_devices=NCORES,
        name="biased_cross_decoder",
    )

    d = {}
    d["ones_t"] = nc.dram_tensor("ones_t", [P, P], F32R, kind="ExternalInput").ap()
    d["cb"] = nc.dram_tensor("cb", [P, P + 64], BF16, kind="ExternalInput").ap()
    d["xT"] = nc.dram_tensor("xT", [D, Q], BF16, kind="ExternalInput").ap()
    d["zT"] = nc.dram_tensor("zT", [D, S], BF16, kind="ExternalInput").ap()
    d["xq"] = nc.dram_tensor("xq", [D, QS], F32, kind="ExternalInput").ap()
    d["maskT"] = nc.dram_tensor("maskT", [2, 2, S, 2, QC], BF16,
                                kind="ExternalInput").ap()
    d["wqT"] = nc.dram_tensor("wqT", [P, KX, FC], BF16, kind="ExternalInput").ap()
    d["wkT"] = nc.dram_tensor("wkT", [P, KX, FC], BF16, kind="ExternalInput").ap()
    d["wvT"] = nc.dram_tensor("wvT", [P, KX, FC], BF16, kind="ExternalInput").ap()
    d["adjq"] = nc.dram_tensor("adjq", [2, FC], F32R, kind="ExternalInput").ap()
    d["adjk"] = nc.dram_tensor("adjk", [2, FC], F32R, kind="ExternalInput").ap()
    d["adjv"] = nc.dram_tensor("adjv", [2, FC], F32R, kind="ExternalInput").ap()
    d["owp"] = nc.dram_tensor("owp", [P, 2, D], BF16, kind="ExternalInput").ap()
    d["outbc"] = nc.dram_tensor("outbc", [P, KX], F32, kind="ExternalInput").ap()
    d["b1c"] = nc.dram_tensor("b1c", [P, FFP], F32, kind="ExternalInput").ap()
    d["b2c"] = nc.dram_tensor("b2c", [P, KX], F32, kind="ExternalInput").ap()
    d["w1h"] = nc.dram_tensor("w1h", [P, FFP, KX, P], BF16,
                              kind="ExternalInput").ap()
    d["w2h"] = nc.dram_tensor("w2h", [P, FFP, D], BF16,
                              kind="ExternalInput").ap()
    d["out"] = nc.dram_tensor("out", [D, QS], F32, kind="ExternalOutput").ap()

    with tile.TileContext(nc) as tc:
        build_tile_program(tc, nc, d)
    nc.compile()
    return nc


class _Pool:
    """Keeps the tile_pool context manager alive; allows explicit close."""

    def __init__(self, cm):
        self._cm = cm
        self.pool = cm.__enter__()

    def tile(self, *a, **kw):
        kw.setdefault("name", kw.get("tag") or "t")
        return self.pool.tile(*a, **kw)

    def close(self):
        self._cm.__exit__(None, None, None)


def build_tile_program(tc, nc, d):
    # ---------------- persistent constants ----------------
    const = _Pool(tc.tile_pool(name="const", bufs=1))
    dram = _Pool(tc.tile_pool(name="dram", bufs=1, space="DRAM"))

    ones_sb = const.tile([P, P], F32R, tag="ones_sb")
    nc.sync.dma_start(ones_sb, d["ones_t"])
    cb_sb = const.tile([P, P + 64], BF16, tag="cb_sb")
    nc.sync.dma_start(cb_sb, d["cb"])
    ident = cb_sb[:, 0:P]            # bf16 identity (mask-add matmul lhsT)
    ones_bcol = cb_sb[:, P:P + 1]    # bf16 ones column (bf16 stat sums)
    ones_col = ones_sb[:, 0:1]       # f32r ones column (fp32 stat sums)
    ones_row = ones_sb[0:1, :]       # f32r row (partition broadcasts)

    eps_t = const.tile([1, 1], F32, tag="eps")
    nc.vector.memset(eps_t, EPS)
    outb_col = const.tile([P, KX], F32, tag="outb_col")
    nc.sync.dma_start(outb_col, d["outbc"])
    b2_col = const.tile([P, KX], F32, tag="b2_col")
    nc.sync.dma_start(b2_col, d["b2c"])
    b1_col = const.tile([P, FFP], F32, tag="b1_col")
    nc.sync.dma_start(b1_col, d["b1c"])
    rz_col = const.tile([P, NST], F32R, tag="rz_col")

    rs_scr = dram.tile([1, S], F32R, tag="rs_scr")
    rs_in = [dram.tile([4, D, P], BF16, name=f"rs_in{i}", tag=f"rs_in{i}")
             for i in range(2)]
    rs_out = [dram.tile([D, P], BF16, name=f"rs_out{i}", tag=f"rs_out{i}")
              for i in range(2)]

    # ---------------- long-lived right-side pools ----------------
    pool_qkv = _Pool(tc.tile_pool(name="qkv", bufs=1, side="right"))
    qT = pool_qkv.tile([P, 2, Q], BF16, tag="qT")
    kT = pool_qkv.tile([P, 2, S], BF16, tag="kT")
    v_sb = pool_qkv.tile([P, NST, NH, HD + 1], BF16, tag="v_sb")
    ow_sb = pool_qkv.tile([P, 2, D], BF16, tag="ow_sb")
    nc.sync.dma_start(ow_sb, d["owp"])

    # ---------------- phase A scratch (left stack) ----------------
    pool_x = _Pool(tc.tile_pool(name="px", bufs=1))
    pool_z = _Pool(tc.tile_pool(name="pz", bufs=1))
    pool_w = _Pool(tc.tile_pool(name="pw", bufs=1))
    pool_adj = _Pool(tc.tile_pool(name="adj", bufs=1))
    pool_bc = _Pool(tc.tile_pool(name="bc", bufs=1))
    pool_sq = _Pool(tc.tile_pool(name="sq", bufs=3))
    pool_rows = _Pool(tc.tile_pool(name="rows", bufs=2))

    xT = pool_x.tile([P, KX, Q], BF16, tag="xT")
    for k in range(KX):
        nc.sync.dma_start(xT[:, k, :], d["xT"][k * P:(k + 1) * P, :])
    zT = pool_z.tile([P, KX, S], BF16, tag="zT")
    for h2 in range(2):
        for k in range(KX):
            nc.scalar.dma_start(zT[:, k, h2 * 1024:(h2 + 1) * 1024],
                                d["zT"][k * P:(k + 1) * P, h2 * 1024:(h2 + 1) * 1024])
    wq_sb = pool_w.tile([P, KX, FC], BF16, tag="wq_sb")
    nc.sync.dma_start(wq_sb, d["wqT"])
    wk_sb = pool_w.tile([P, KX, FC], BF16, tag="wk_sb")
    nc.sync.dma_start(wk_sb, d["wkT"])
    wv_sb = pool_w.tile([P, KX, FC], BF16, tag="wv_sb")
    nc.sync.dma_start(wv_sb, d["wvT"])
    adjq_w = pool_w.tile([2, FC], F32R, tag="adjq_w")
    nc.sync.dma_start(adjq_w, d["adjq"])
    adjk_w = pool_w.tile([2, FC], F32R, tag="adjk_w")
    nc.sync.dma_start(adjk_w, d["adjk"])
    adjv_w = pool_w.tile([2, FC], F32R, tag="adjv_w")
    nc.sync.dma_start(adjv_w, d["adjv"])

    adjx = pool_adj.tile([2, Q], F32R, tag="adjx")      # [mean ; std] rows
    adjz = pool_adj.tile([2, S], F32R, tag="adjz")
    rxB = pool_bc.tile([P, Q], F32, tag="rxB")          # 1/std broadcast
    rzB = pool_bc.tile([P, S], F32, tag="rzB")

    def ln_stats(aT, T, adj, rB, ps_stats, scr=None):
        """Per 512-token chunk: LN stats -> adj=[mean;std] rows and a
        [P, T] broadcast of 1/std (via gpsimd partition_broadcast)."""
        for ch in range(T // 512):
            sl = slice(ch * 512, (ch + 1) * 512)
            ps_sum = ps_stats.tile([1, 512], F32, name="ps_sum", tag="ps_sum")
            ps_ssq = ps_stats.tile([1, 512], F32, name="ps_ssq", tag="ps_ssq")
            for k in range(KX):
                nc.tensor.matmul(ps_sum, ones_bcol, aT[:, k, sl],
                                 start=(k == 0), stop=(k == KX - 1))
                sq = pool_sq.tile([P, 512], BF16, name="sq", tag="sq")
                nc.scalar.square(sq, aT[:, k, sl])
                nc.tensor.matmul(ps_ssq, ones_bcol, sq,
                                 start=(k == 0), stop=(k == KX - 1))
            e2 = pool_rows.tile([1, 512], F32, name="e2", tag="e2")
            m2 = pool_rows.tile([1, 512], F32, name="m2", tag="m2")
            inv = pool_rows.tile([1, 512], F32R, name="inv", tag="inv")
            rr = pool_rows.tile([1, 512], F32R, name="rr", tag="rr")
            nc.vector.tensor_scalar_mul(adj[0:1, sl], ps_sum, 1.0 / D)  # mean
            nc.vector.tensor_scalar_mul(e2, ps_ssq, 1.0 / D)            # E[x^2]
            nc.vector.tensor_mul(m2, adj[0:1, sl], adj[0:1, sl])
            nc.vector.tensor_sub(e2, e2, m2)                            # var
            nc.scalar.activation(inv, e2, AF.Sqrt, bias=eps_t[0:1])     # std
            nc.vector.reciprocal_approx_fast(_f(rr), _f(inv))
            nc.scalar.dma_start(adj[1:2, sl], inv)   # cross-partition row move
            nc.gpsimd.partition_broadcast(rB[:, sl], _f(rr))
            if scr is not None:
                nc.scalar.dma_start(scr[0:1, sl], rr)

    # ---- x statistics + q projection ----
    with tc.tile_pool(name="ps_sx", bufs=2, space="PSUM") as ps_sx:
        ln_stats(xT, Q, adjx, rxB, ps_sx)

    with tc.tile_pool(name="ps_q", bufs=3, space="PSUM") as ps_qk:
        for m in range(2):
            for ch in range(2):
                sl = slice(ch * 512, (ch + 1) * 512)
                ps = ps_qk.tile([P, 512], F32, name="ps_qk_t", tag="ps_qk_t")
                for k in range(KX):
                    nc.tensor.matmul(ps, wq_sb[:, k, m * P:(m + 1) * P],
                                     xT[:, k, sl], start=(k == 0), stop=False)
                nc.tensor.matmul(ps, adjq_w[:, m * P:(m + 1) * P],
                                 _r(adjx[:, sl]), start=False, stop=True)
                nc.vector.tensor_mul(qT[:, m, sl], ps, rxB[:, sl])

    # ---- z statistics + k/v projections ----
    with tc.tile_pool(name="ps_sz", bufs=2, space="PSUM") as ps_sz:
        ln_stats(zT, S, adjz, rzB, ps_sz, scr=rs_scr)
    nc.scalar.dma_start(rz_col, rs_scr.rearrange("a (i p) -> (a p) i", p=P))

    # softmax-denominator ones column
    nc.sync.dma_start(
        v_sb[:, :, :, HD:HD + 1],
        d["cb"][:, P:P + 64].rearrange("p (a b c) -> p a b c", a=NST, c=1))

    with tc.tile_pool(name="ps_k", bufs=3, space="PSUM") as ps_qk, \
         tc.tile_pool(name="ps_v", bufs=2, space="PSUM") as ps_v:
        for m in range(2):
            for ch in range(4):
                sl = slice(ch * 512, (ch + 1) * 512)
                ps = ps_qk.tile([P, 512], F32, name="ps_qk_t", tag="ps_qk_t")
                for k in range(KX):
                    nc.tensor.matmul(ps, wk_sb[:, k, m * P:(m + 1) * P],
                                     zT[:, k, sl], start=(k == 0), stop=False)
                nc.tensor.matmul(ps, adjk_w[:, m * P:(m + 1) * P],
                                 _r(adjz[:, sl]), start=False, stop=True)
                nc.vector.tensor_mul(kT[:, m, sl], ps, rzB[:, sl])

        for t in range(NST):
            ps = ps_v.tile([P, FC], F32, name="ps_v_t", tag="ps_v_t")
            for k in range(KX):
                nc.tensor.matmul(ps, zT[:, k, t * P:(t + 1) * P],
                                 wv_sb[:, k, :], start=(k == 0), stop=False)
            nc.tensor.matmul(ps, _r(adjz[:, t * P:(t + 1) * P]), _r(adjv_w),
                             start=False, stop=True)
            nc.vector.tensor_scalar_mul(
                v_sb[:, t, :, 0:HD],
                ps.rearrange("p (h e) -> p h e", h=NH),
                _f(rz_col[:, t:t + 1]))

    pool_rows.close()
    pool_sq.close()
    pool_bc.close()
    pool_adj.close()
    pool_w.close()
    pool_z.close()
    pool_x.close()

    # ---------------- resident FFN weights (prefetched during attention) ----
    pool_w1 = _Pool(tc.tile_pool(name="w1r", bufs=1))
    w1sb = pool_w1.tile([P, FFP, KX, P], BF16, tag="w1sb")
    w2sb = pool_w1.tile([P, FFP, D], BF16, tag="w2sb")
    for j0 in range(0, FFP, 8):
        nc.gpsimd.dma_start(w1sb[:, j0:j0 + 8], d["w1h"][:, j0:j0 + 8])
        nc.gpsimd.dma_start(w2sb[:, j0:j0 + 8], d["w2h"][:, j0:j0 + 8])

    # =================== attention (query-chunked) ===================
    pool_att2 = _Pool(tc.tile_pool(name="att2", bufs=1))
    pool_mk = _Pool(tc.tile_pool(name="mk", bufs=9))
    pool_pr0 = _Pool(tc.tile_pool(name="pr0", bufs=3))
    pool_pr1 = _Pool(tc.tile_pool(name="pr1", bufs=3))
    pool_nrm = _Pool(tc.tile_pool(name="nrm", bufs=2))
    pool_osb = _Pool(tc.tile_pool(name="osb", bufs=3))

    for ci in range(2):
        qsl = slice(ci * QC, (ci + 1) * QC)
        att2 = [pool_att2.tile([P, QC], BF16, name=f"att2_{p}", tag=f"att2_{p}")
                for p in range(2)]

        ps_lg0_cm = tc.tile_pool(name="ps_lg0", bufs=3, space="PSUM")
        ps_lg1_cm = tc.tile_pool(name="ps_lg1", bufs=3, space="PSUM")
        ps_att_cm = tc.tile_pool(name="ps_att", bufs=1, space="PSUM")
        ps_lg0 = ps_lg0_cm.__enter__()
        ps_lg1 = ps_lg1_cm.__enter__()
        ps_att = ps_att_cm.__enter__()

        for pair in range(2):
            attps = [ps_att.tile([HD + 1, QC], F32, name=f"attps{hh}",
                                 tag=f"attps{hh}") for hh in range(2)]

            def emit_pv(st, p0, p1):
                nc.tensor.matmul(attps[0], v_sb[:, st, 2 * pair, :], p0,
                                 start=(st == 0), stop=(st == NST - 1))
                nc.tensor.matmul(attps[1], v_sb[:, st, 2 * pair + 1, :], p1,
                                 start=(st == 0), stop=(st == NST - 1))

            prev = None
            for st in range(NST):
                ssl = slice(st * P, (st + 1) * P)
                mk = pool_mk.tile([P, 2, QC], BF16, name="mk", tag="mk")
                if st % 2 == 0:
                    nc.sync.dma_start(mk, d["maskT"][ci, pair, ssl, :, :])
                else:
                    nc.gpsimd.dma_start(mk, d["maskT"][ci, pair, ssl, :, :])
                lg0 = ps_lg0.tile([P, QC], F32, name="lg0", tag="lg0")
                lg1 = ps_lg1.tile([P, QC], F32, name="lg1", tag="lg1")
                # head-pair QK in two concurrent 64-row PE groups
                nc.tensor.matmul(lg0, kT[0:HD, pair, ssl], qT[0:HD, pair, qsl],
                                 start=True, stop=False)
                nc.tensor.matmul(lg1, kT[HD:P, pair, ssl], qT[HD:P, pair, qsl],
                                 start=True, stop=False)
                # mask add via identity matmul into the same PSUM group
                nc.tensor.matmul(lg0, ident, mk[:, 0, :], start=False, stop=True)
                nc.tensor.matmul(lg1, ident, mk[:, 1, :], start=False, stop=True)
                pr0 = pool_pr0.tile([P, QC], BF16, name="pr0", tag="pr0")
                nc.scalar.activation(pr0, lg0, AF.Exp)
                pr1 = pool_pr1.tile([P, QC], BF16, name="pr1", tag="pr1")
                nc.scalar.activation(pr1, lg1, AF.Exp)
                if prev is not None:
                    emit_pv(*prev)
                prev = (st, pr0, pr1)
            emit_pv(*prev)

            # normalize: att2[pair][64h:64h+64] = attps[h][0:64] / attps[h][64]
            for hh in range(2):
                den = pool_nrm.tile([1, QC], F32, name="den", tag="den")
                nc.vector.tensor_copy(den, attps[hh][HD:HD + 1, :])
                r0 = pool_nrm.tile([1, QC], F32, name="r0", tag="r0")
                nc.vector.reciprocal_approx_fast(r0, den)
                nbc = pool_nrm.tile([HD, QC], F32, name="nbc", tag="nbc")
                nc.gpsimd.partition_broadcast(nbc, r0)
                nc.vector.tensor_mul(att2[pair][HD * hh:HD * hh + HD, :],
                                     attps[hh][0:HD, :], nbc)

        ps_att_cm.__exit__(None, None, None)
        ps_lg1_cm.__exit__(None, None, None)
        ps_lg0_cm.__exit__(None, None, None)

        # ---- out-projection for this chunk + ReduceScatter ----
        with tc.tile_pool(name="ps_o", bufs=2, space="PSUM") as ps_o:
            for m in range(KX):
                ps = ps_o.tile([P, QC], F32, name="ps_o_t", tag="ps_o_t")
                nc.tensor.matmul(ps, ow_sb[:, 0, m * P:(m + 1) * P], att2[0],
                                 start=True, stop=False)
                nc.tensor.matmul(ps, ow_sb[:, 1, m * P:(m + 1) * P], att2[1],
                                 start=False, stop=True)
                ot = pool_osb.tile([P, QC], BF16, name="ot", tag="ot")
                nc.vector.tensor_copy(ot, ps)
                nc.scalar.dma_start(
                    rs_in[ci][:, m * P:(m + 1) * P, :].rearrange(
                        "r p q -> p r q"),
                    ot.rearrange("p (r q) -> p r q", r=4))
        nc.gpsimd.collective_compute(
            "ReduceScatter",
            ALU.add,
            replica_groups=REPLICA_GROUPS,
            ins=[rs_in[ci].opt()],
            outs=[rs_out[ci].opt()],
        )

    pool_osb.close()
    pool_nrm.close()
    pool_pr1.close()
    pool_pr0.close()
    pool_mk.close()
    pool_att2.close()
    pool_qkv.close()

    # =================== residual + FFN (sequence-parallel) ===================
    pool_f = _Pool(tc.tile_pool(name="ffn", bufs=1, side="right"))
    pool_rsld = _Pool(tc.tile_pool(name="rsld", bufs=3))
    pool_fsq = _Pool(tc.tile_pool(name="fsq", bufs=2))
    pool_frow = _Pool(tc.tile_pool(name="frow", bufs=2))
    pool_ftmp = _Pool(tc.tile_pool(name="ftmp", bufs=2))
    pool_yo = _Pool(tc.tile_pool(name="yout", bufs=3))

    y1T = pool_f.tile([P, KX, QS], F32R, tag="y1T")
    y1n = pool_f.tile([P, KX, QS], BF16, tag="y1n")
    g_sb = pool_f.tile([P, FFP, QS], BF16, tag="g_sb")
    m_row = pool_f.tile([1, QS], F32R, tag="m_row")
    r_row = pool_f.tile([1, QS], F32, tag="r_row")
    r_rowR = pool_f.tile([1, QS], F32R, tag="r_rowR")
    myB = pool_f.tile([P, QS], F32, tag="myB")
    ryB = pool_f.tile([P, QS], F32, tag="ryB")
    xq_sb = pool_f.tile([P, KX, QS], F32, tag="xq_sb")
    nc.gpsimd.dma_start(xq_sb, d["xq"].rearrange("(k p) q -> p k q", p=P))

    def piece_prep(piece):
        """y1 assembly + LN stats + normalize for one 128-query piece."""
        psl = slice(piece * P, (piece + 1) * P)
        for k in range(KX):
            rst = pool_rsld.tile([P, P], BF16, name="rst", tag="rst")
            nc.gpsimd.dma_start(rst, rs_out[piece][k * P:(k + 1) * P, :])
            nc.vector.scalar_tensor_tensor(
                out=y1T[:, k, psl], in0=rst, scalar=outb_col[:, k:k + 1],
                in1=xq_sb[:, k, psl], op0=ALU.add, op1=ALU.add)
        with tc.tile_pool(name="ps_yst", bufs=1, space="PSUM") as ps_yst, \
             tc.tile_pool(name="ps_yb", bufs=1, space="PSUM") as ps_yb:
            ps_sum = ps_yst.tile([1, P], F32, name="ps_sum2", tag="ps_sum2")
            ps_ssq = ps_yst.tile([1, P], F32, name="ps_ssq2", tag="ps_ssq2")
            for k in range(KX):
                nc.tensor.matmul(ps_sum, _r(ones_col), y1T[:, k, psl],
                                 start=(k == 0), stop=(k == KX - 1))
                sqy = pool_fsq.tile([P, P], F32R, name="sqy", tag="sqy")
                nc.scalar.square(sqy, y1T[:, k, psl])
                nc.tensor.matmul(ps_ssq, _r(ones_col), _r(sqy),
                                 start=(k == 0), stop=(k == KX - 1))
            e2 = pool_frow.tile([1, P], F32, name="e2y", tag="e2y")
            m2 = pool_frow.tile([1, P], F32, name="m2y", tag="m2y")
            inv = pool_frow.tile([1, P], F32, name="invy", tag="invy")
            nc.vector.tensor_scalar_mul(m_row[0:1, psl], ps_sum, 1.0 / D)
            nc.vector.tensor_scalar_mul(e2, ps_ssq, 1.0 / D)
            nc.vector.tensor_mul(m2, _f(m_row[0:1, psl]), _f(m_row[0:1, psl]))
            nc.vector.tensor_sub(e2, e2, m2)
            nc.scalar.activation(inv, e2, AF.Sqrt, bias=eps_t[0:1])
            nc.vector.reciprocal_approx_fast(r_row[0:1, psl], inv)
            nc.vector.tensor_copy(r_rowR[0:1, psl], r_row[0:1, psl])
            bcm = ps_yb.tile([P, P], F32, name="bcm", tag="bcm")
            nc.tensor.matmul(bcm, _r(ones_row), m_row[0:1, psl])
            nc.vector.tensor_copy(myB[:, psl], bcm)
            bcr = ps_yb.tile([P, P], F32, name="bcr", tag="bcr")
            nc.tensor.matmul(bcr, _r(ones_row), r_rowR[0:1, psl])
            nc.vector.tensor_copy(ryB[:, psl], bcr)
        for k in range(KX):
            tmp = pool_ftmp.tile([P, P], F32, name="tmpn", tag="tmpn")
            nc.vector.tensor_sub(tmp, y1T[:, k, psl], myB[:, psl])
            nc.vector.tensor_mul(y1n[:, k, psl], tmp, ryB[:, psl])

    def ff1_piece(piece, ps_f, y2a=None):
        """ff1 + gelu for one piece; optionally interleave ff2 pass 1."""
        psl = slice(piece * P, (piece + 1) * P)
        for j in range(FFP):
            ps = ps_f.tile([P, P], F32, name="ps_f_t", tag="ps_f_t")
            for k in range(KX):
                nc.tensor.matmul(ps, w1sb[:, j, k, :], y1n[:, k, psl],
                                 start=(k == 0), stop=(k == KX - 1))
            nc.scalar.activation(g_sb[:, j, psl], ps, AF.Gelu,
                                 bias=b1_col[:, j:j + 1])
            if y2a is not None:
                for mi in range(4):
                    nc.tensor.matmul(y2a[mi], w2sb[:, j, mi * P:(mi + 1) * P],
                                     g_sb[:, j, :],
                                     start=(j == 0), stop=(j == FFP - 1))

    with tc.tile_pool(name="ps_f", bufs=2, space="PSUM") as ps_f:

        def emit_y2(y2a, mi):
            yt = pool_yo.tile([P, QS], F32, name="yt", tag="yt")
            nc.vector.scalar_tensor_tensor(
                out=yt, in0=y2a[mi % 4], scalar=b2_col[:, mi:mi + 1],
                in1=y1T[:, mi, :], op0=ALU.add, op1=ALU.add)
            nc.scalar.dma_start(d["out"][mi * P:(mi + 1) * P, :], yt)

        # piece A: runs while the chunk-B ReduceScatter is still in flight
        tc.no_sync_barrier()
        piece_prep(0)
        ff1_piece(0, ps_f)
        # piece B + ff2 pass 1 (needs both pieces' activations per block)
        tc.no_sync_barrier()
        piece_prep(1)
        ps_y2_cm = tc.tile_pool(name="ps_y2", bufs=1, space="PSUM")
        ps_y2 = ps_y2_cm.__enter__()
        y2a = [ps_y2.tile([P, QS], F32, name=f"y2a_{i}", tag=f"y2a_{i}")
               for i in range(4)]
        ff1_piece(1, ps_f, y2a)
        for mi in range(4):
            emit_y2(y2a, mi)
        # ff2 pass 2 over the resident activations
        y2a = [ps_y2.tile([P, QS], F32, name=f"y2a_{i}", tag=f"y2a_{i}")
               for i in range(4)]
        for j in range(FFP):
            for mi in range(4, KX):
                nc.tensor.matmul(y2a[mi - 4], w2sb[:, j, mi * P:(mi + 1) * P],
                                 g_sb[:, j, :],
                                 start=(j == 0), stop=(j == FFP - 1))
        for mi in range(4, KX):
            emit_y2(y2a, mi)
        ps_y2_cm.__exit__(None, None, None)

    pool_yo.close()
    pool_ftmp.close()
    pool_frow.close()
    pool_fsq.close()
    pool_rsld.close()
    pool_f.close()
    pool_w1.close()
    const.close()
    dram.close()


def host_prep(inputs):
    """Fold layernorm gains/biases into (bf16) weights; build per-core shards."""
    f32 = np.float32
    bf = ml_dtypes.bfloat16
    x = np.asarray(inputs["x"], f32)
    z = np.asarray(inputs["z"], f32)
    mask = np.asarray(inputs["attn_mask"], f32)
    gq = np.asarray(inputs["gq"], np.float64)
    bq = np.asarray(inputs["bq"], np.float64)
    gkv = np.asarray(inputs["gkv"], np.float64)
    bkv = np.asarray(inputs["bkv"], np.float64)
    gff = np.asarray(inputs["gff"], np.float64)
    bff = np.asarray(inputs["bff"], np.float64)
    ipw = np.asarray(inputs["in_proj_w"], np.float64)
    ipb = np.asarray(inputs["in_proj_b"], np.float64)
    out_w = np.asarray(inputs["out_w"], f32)
    out_b = np.asarray(inputs["out_b"], f32)
    w1 = np.asarray(inputs["w1"], np.float64)
    b1 = np.asarray(inputs["b1"], np.float64)
    w2 = np.asarray(inputs["w2"], f32)
    b2 = np.asarray(inputs["b2"], f32)

    wq, wk, wv = ipw[:D], ipw[D:2 * D], ipw[2 * D:]
    pq, pk, pv = ipb[:D], ipb[D:2 * D], ipb[2 * D:]
    scale = 1.0 / np.sqrt(HD)
    wq2 = ((wq * gq[None, :]) * scale).astype(bf)
    pq2 = ((wq @ bq + pq) * scale).astype(f32)
    wk2 = (wk * gkv[None, :]).astype(bf)
    pk2 = (wk @ bkv + pk).astype(f32)
    wv2 = (wv * gkv[None, :]).astype(bf)
    pv2 = (wv @ bkv + pv).astype(f32)
    # rowsums of the *rounded* weights so the mean correction is consistent
    wq2r = wq2.astype(np.float64)
    wk2r = wk2.astype(np.float64)
    wv2r = wv2.astype(np.float64)

    w1b = (w1 * gff[None, :]).astype(bf)
    b12 = (w1b.astype(np.float64) @ bff + b1).astype(f32)
    b1c = np.ascontiguousarray(b12.reshape(FFP, P).T)
    w1T = np.ascontiguousarray(w1b.T)                              # (D, FF)
    # [p, j, k, o]: each partition's (j,k,o) block is contiguous in DRAM
    w1h = np.ascontiguousarray(
        w1T.reshape(KX, P, FFP, P).transpose(1, 2, 0, 3))
    w2T = np.ascontiguousarray(w2.T.astype(bf))                    # (FF, D)
    w2h = np.ascontiguousarray(
        w2T.reshape(FFP, P, D).transpose(1, 0, 2))                 # (P, FFP, D)

    def pack_kxf(wT):  # (D, FC) bf16 -> (P, D//P, FC)
        return np.ascontiguousarray(wT.reshape(KX, P, FC).transpose(1, 0, 2))

    cb = np.concatenate(
        [np.eye(P, dtype=f32), np.ones((P, 64), f32)], axis=1).astype(bf)

    in_maps = []
    for c in range(NCORES):
        b, hg = c // 4, c % 4
        fs = slice(FC * hg, FC * hg + FC)
        qidx = np.r_[P * hg:P * hg + P, 512 + P * hg:512 + P * hg + P]
        xTb = np.ascontiguousarray(x[b].T)                         # (D, Q)
        mk = mask[16 * b + NH * hg:16 * b + NH * hg + NH]          # (NH, Q, S)
        mkT = mk.transpose(0, 2, 1)                                # (NH, S, Q)
        m5 = mkT.reshape(2, 2, S, Q).transpose(0, 2, 1, 3)         # (pair,S,hh,Q)
        maskT = np.ascontiguousarray(
            np.stack([m5[:, :, :, 0:QC], m5[:, :, :, QC:]],
                     axis=0)).astype(bf)                           # (ci,pair,S,hh,QC)
        in_maps.append({
            "ones_t": np.ones((P, P), f32),
            "cb": cb,
            "xT": xTb.astype(bf),
            "zT": np.ascontiguousarray(z[b].T).astype(bf),
            "xq": np.ascontiguousarray(xTb[:, qidx]),
            "maskT": maskT,
            "wqT": pack_kxf(np.ascontiguousarray(wq2[fs].T)),
            "wkT": pack_kxf(np.ascontiguousarray(wk2[fs].T)),
            "wvT": pack_kxf(np.ascontiguousarray(wv2[fs].T)),
            "adjq": np.ascontiguousarray(
                np.stack([-wq2r[fs].sum(1), pq2[fs]]).astype(f32)),
            "adjk": np.ascontiguousarray(
                np.stack([-wk2r[fs].sum(1), pk2[fs]]).astype(f32)),
            "adjv": np.ascontiguousarray(
                np.stack([-wv2r[fs].sum(1), pv2[fs]]).astype(f32)),
            "owp": np.ascontiguousarray(
                out_w[:, fs].T.reshape(2, P, D).transpose(1, 0, 2)).astype(bf),
            "outbc": np.ascontiguousarray(out_b.reshape(KX, P).T),
            "b1c": b1c,
            "b2c": np.ascontiguousarray(b2.reshape(KX, P).T),
            "w1h": w1h,
            "w2h": w2h,
        })
    return in_maps


_NC_CACHE = None


def kernel(**inputs) -> np.ndarray:
    global _NC_CACHE, LAST_RESULT
    from concourse.bass_utils import run_bass_kernel_spmd

    in_maps = host_prep(inputs)
    if _NC_CACHE is None:
        _NC_CACHE = build_nc()
    res = run_bass_kernel_spmd(
        _NC_CACHE, in_maps, core_ids=list(range(NCORES)),
        trace=bool(os.environ.get("BASS_TRACE")),
    )
    LAST_RESULT = res
    out = np.empty((B, Q, D), np.float32)
    for c in range(NCORES):
        b, hg = c // 4, c % 4
        yT = res.results[c]["out"]                    # (D, QS)
        out[b, P * hg:P * hg + P, :] = yT[:, 0:P].T
        out[b, 512 + P * hg:512 + P * hg + P, :] = yT[:, P:2 * P].T
    return out
